# revision 15
# baseline (speedup 1.0000x reference)
# Trainium2 Bass kernel for nn_Attention_10342281248904 (sparse_attention).
#
# Sharding: tensor-parallel over heads H=8, one head per NeuronCore.
# Each core: q/k/v projections for its head, the hnijd Gram contraction,
# softmax + network-bias branch, context matmul. The mean-over-heads in
# q1_proj is algebraically collapsed to a single weighted d-contraction
# ("a" channel) and realized with one AllReduce; the output projection is
# done after an AllToAll that gives each core a 16-row slice of the full
# 512-channel context (row-sliced data parallel out_proj, no final
# all-reduce needed).
import math
import os
import numpy as np
import ml_dtypes

import concourse.bass as bass
import concourse.mybir as mybir
import concourse.tile as tile
from concourse import bacc
from concourse import bass_utils

# Problem constants (hardcoded per task contract)
R, N, B, E, H, M = 128, 256, 1, 512, 8, 4
D = E // H          # 64 head dim
NCORES = 8
RS = R // NCORES    # 16 rows of R per core in the output slice
P = 128
KT = E // P         # 4 contraction tiles for E
NEG = -1.0e9
SCALING = (D ** -0.5) / math.sqrt(R)

FP32 = mybir.dt.float32
F32R = mybir.dt.float32r
BF16 = mybir.dt.bfloat16

BF16_NP = ml_dtypes.bfloat16


def build_program():
    """Build the SPMD Bass program (same NEFF on all 8 cores; per-core
    behavior differs only through per-core input tensors)."""
    nc = bacc.Bacc(
        "TRN2",
        target_bir_lowering=False,
        debug=False,
        num_devices=NCORES,
    )

    # ---- I/O ----
    xT = nc.dram_tensor("xT", [E, N, R], BF16, kind="ExternalInput")
    w4 = nc.dram_tensor("w4", [E, 256], BF16, kind="ExternalInput")
    wo_t = nc.dram_tensor("wo_t", [E, E], BF16, kind="ExternalInput")
    lrep = nc.dram_tensor("lrep", [P, 1], FP32, kind="ExternalInput")
    negeye = nc.dram_tensor("negeye", [2, P, N], FP32, kind="ExternalInput")
    net = nc.dram_tensor("net", [N, N, M], FP32, kind="ExternalInput")

    probs_out = nc.dram_tensor("probs_out", [N, N], FP32, kind="ExternalOutput")
    out_slice = nc.dram_tensor("out_slice", [RS * N, E], FP32, kind="ExternalOutput")

    xT_ap = xT.ap()
    rg = [list(range(NCORES))]

    with tile.TileContext(nc) as tc:
        with (
            tc.tile_pool(name="const", bufs=1) as const_pool,
            tc.tile_pool(name="big", bufs=1) as big_pool,
            tc.tile_pool(name="dram", bufs=1, space="DRAM") as dram_pool,
            tc.tile_pool(name="sm", bufs=2) as sm_pool,
        ):
            # Persistent SBUF tensors
            w4_sb = const_pool.tile([P, KT, 256], BF16, name="w4_sb")
            nc.sync.dma_start(w4_sb, w4.ap().rearrange("(kt p) c -> p kt c", p=P))
            ident = const_pool.tile([P, P], BF16, name="ident")
            from concourse.masks import make_identity
            make_identity(nc, ident)
            lrep_sb = const_pool.tile([P, 1], FP32, name="lrep_sb")
            nc.sync.dma_start(lrep_sb, lrep.ap())

            # DRAM bounce buffers for collectives
            ar_in = dram_pool.tile([N, N], FP32, name="ar_in")
            ar_out = dram_pool.tile([N, N], FP32, name="ar_out", addr_space="Shared")
            a2a_in = dram_pool.tile([R, D, N], BF16, name="a2a_in")
            a2a_out = dram_pool.tile([NCORES, RS, D, N], BF16, name="a2a_out")

            # ---------------- Phases 1+2 share the big qkv tensor ----------------
            qkv_pool_cm = tc.tile_pool(name="qkvp", bufs=1)
            qkv_pool = qkv_pool_cm.__enter__()
            # qkv: [r, (i, ch)] with ch = [q(0:64) | k(64:128) | qa(128:192) | v(192:256)]
            qkv = qkv_pool.tile([P, N * 256], BF16, name="qkv")
            qkv_r = qkv.rearrange("p (i c) -> p i c", c=256)

            # ---------------- Phase 1: projections ----------------
            IB = 16  # i-block streamed per DMA
            with (
                tc.tile_pool(name="xt", bufs=2) as xt_pool,
                tc.tile_pool(name="ps1", bufs=4, space="PSUM") as ps1_pool,
            ):
                for ib in range(N // IB):
                    xts = []
                    for kt in range(KT):
                        t = xt_pool.tile(
                            [P, IB * P], BF16, name=f"xt{kt}", tag=f"xt{kt}"
                        )
                        nc.sync.dma_start(
                            t,
                            xT_ap[
                                kt * P : (kt + 1) * P, ib * IB : (ib + 1) * IB, :
                            ].rearrange("e i r -> e (i r)"),
                        )
                        xts.append(t)
                    for ii in range(0, IB, 2):
                        ps = ps1_pool.tile([P, 512], FP32, name="ps1", tag="ps1")
                        for half in range(2):
                            i_loc = ii + half
                            for kt in range(KT):
                                nc.tensor.matmul(
                                    ps[:, half * 256 : (half + 1) * 256],
                                    lhsT=xts[kt][:, i_loc * P : (i_loc + 1) * P],
                                    rhs=w4_sb[:, kt],
                                    start=(kt == 0),
                                    stop=(kt == KT - 1),
                                )
                        i_glob = ib * IB + ii
                        nc.vector.tensor_copy(
                            out=qkv[:, i_glob * 256 : (i_glob + 2) * 256],
                            in_=ps,
                        )

            # ---------------- Phase 2: Gram contractions + v transpose ----------------
            # v_T[jc]: [j (128), (r, d)] bf16 for the context matmul
            v_T = [
                big_pool.tile([P, R * D], BF16, name=f"v_T{jc}") for jc in range(2)
            ]
            attn_sb = [
                sm_pool.tile([P, N], FP32, name=f"attn_sb{ic}", tag=f"attn_sb{ic}")
                for ic in range(2)
            ]
            a_sb = [
                sm_pool.tile([P, N], FP32, name=f"a_sb{ic}", tag=f"a_sb{ic}")
                for ic in range(2)
            ]

            with (
                tc.tile_pool(name="psacc", bufs=1, space="PSUM") as psacc_pool,
                tc.tile_pool(name="pst", bufs=2, space="PSUM") as pst_pool,
            ):
                a_ps = [
                    psacc_pool.tile([P, N], FP32, name=f"a_ps{ic}", tag=f"a_ps{ic}")
                    for ic in range(2)
                ]
                attn_ps = [
                    psacc_pool.tile([P, N], FP32, name=f"at_ps{ic}", tag=f"at_ps{ic}")
                    for ic in range(2)
                ]
                # a-channel first so the AllReduce can start early
                for d in range(D):
                    kr = qkv_r[:, :, 64 + d]
                    for ic in range(2):
                        nc.tensor.matmul(
                            a_ps[ic],
                            lhsT=qkv_r[:, ic * P : (ic + 1) * P, 128 + d],
                            rhs=kr,
                            start=(d == 0),
                            stop=(d == D - 1),
                        )
                for ic in range(2):
                    nc.vector.tensor_copy(out=a_sb[ic], in_=a_ps[ic])
                    nc.sync.dma_start(ar_in[ic * P : (ic + 1) * P, :], a_sb[ic])
                nc.gpsimd.collective_compute(
                    "AllReduce",
                    mybir.AluOpType.add,
                    replica_groups=rg,
                    ins=[ar_in.opt()],
                    outs=[ar_out.opt()],
                )

                # attn channel
                for d in range(D):
                    kr = qkv_r[:, :, 64 + d]
                    for ic in range(2):
                        nc.tensor.matmul(
                            attn_ps[ic],
                            lhsT=qkv_r[:, ic * P : (ic + 1) * P, d],
                            rhs=kr,
                            start=(d == 0),
                            stop=(d == D - 1),
                        )
                for ic in range(2):
                    nc.vector.tensor_copy(out=attn_sb[ic], in_=attn_ps[ic])

                # v transposes: qkv [r, (j, 192+d)] -> v_T[jc] [j, (r, d)]
                for jc in range(2):
                    vtr = v_T[jc].rearrange("p (r d) -> p r d", d=D)
                    for d in range(D):
                        tp = pst_pool.tile([P, P], BF16, name="tp", tag="tp")
                        nc.tensor.transpose(
                            tp, qkv_r[:, jc * P : (jc + 1) * P, 192 + d], ident
                        )
                        nc.scalar.copy(vtr[:, :, d], tp)
            qkv_pool_cm.__exit__(None, None, None)

            # ---------------- Phase 3: network branch + softmax ----------------
            probs_bf = [
                big_pool.tile([P, N], BF16, name=f"probs_bf{ic}") for ic in range(2)
            ]
            with tc.tile_pool(name="ph3", bufs=1) as ph3:
                a_full = [
                    ph3.tile([P, N], FP32, name=f"a_full{ic}", tag=f"a_full{ic}")
                    for ic in range(2)
                ]
                for ic in range(2):
                    nc.sync.dma_start(a_full[ic], ar_out[ic * P : (ic + 1) * P, :])
                for ic in range(2):
                    net_sb = ph3.tile([P, N * M], FP32, name="net_sb", tag="net_sb")
                    nc.sync.dma_start(
                        net_sb,
                        net.ap()[ic * P : (ic + 1) * P].rearrange("i j m -> i (j m)"),
                    )
                    net_r = net_sb.rearrange("p (j m) -> p j m", m=M)
                    na = ph3.tile([P, N * M], FP32, name="na", tag="na")
                    na_r = na.rearrange("p (j m) -> p j m", m=M)
                    a_b = a_full[ic][:, :, None].to_broadcast([P, N, M])
                    nc.vector.tensor_tensor(na_r, net_r, a_b, mybir.AluOpType.mult)
                    # softmax over m (4 channels); |na| << 1 so no max-shift needed
                    ne = ph3.tile([P, N * M], FP32, name="ne", tag="ne")
                    nc.scalar.activation(
                        ne, na, mybir.ActivationFunctionType.Exp
                    )
                    ne_r = ne.rearrange("p (j m) -> p j m", m=M)
                    s4 = ph3.tile([P, N], FP32, name="s4", tag="s4")
                    nc.vector.reduce_sum(s4, ne_r, axis=mybir.AxisListType.X)
                    rinv = ph3.tile([P, N], FP32, name="rinv", tag="rinv")
                    nc.vector.reciprocal(rinv, s4)
                    # net_bias = (sum_m network * exp) / sum_m exp
                    tw = ph3.tile([P, N * M], FP32, name="tw", tag="tw")
                    nc.vector.tensor_tensor(tw, ne, net_sb, mybir.AluOpType.mult)
                    tw_r = tw.rearrange("p (j m) -> p j m", m=M)
                    nb = ph3.tile([P, N], FP32, name="nb", tag="nb")
                    nc.vector.reduce_sum(nb, tw_r, axis=mybir.AxisListType.X)
                    nc.vector.tensor_tensor(nb, nb, rinv, mybir.AluOpType.mult)
                    # attn_final = attn + l * net_bias + negeye
                    lnb = ph3.tile([P, N], FP32, name="lnb", tag="lnb")
                    nc.vector.tensor_scalar_mul(lnb, nb, lrep_sb[:, 0:1])
                    negeye_sb = ph3.tile([P, N], FP32, name="negeye_sb", tag="ney")
                    nc.sync.dma_start(negeye_sb, negeye.ap()[ic])
                    af = ph3.tile([P, N], FP32, name="af", tag="af")
                    nc.vector.tensor_tensor(af, attn_sb[ic], lnb, mybir.AluOpType.add)
                    nc.vector.tensor_tensor(af, af, negeye_sb, mybir.AluOpType.add)
                    # row softmax over j
                    negmx = ph3.tile([P, 1], FP32, name="negmx", tag="negmx")
                    nc.vector.tensor_reduce(
                        negmx, af, axis=mybir.AxisListType.X,
                        op=mybir.AluOpType.max, negate=True,
                    )
                    pex = ph3.tile([P, N], FP32, name="pex", tag="pex")
                    rowsum = ph3.tile([P, 1], FP32, name="rowsum", tag="rowsum")
                    nc.scalar.activation(
                        pex, af, mybir.ActivationFunctionType.Exp,
                        bias=negmx[:, 0:1], accum_out=rowsum[:, 0:1],
                    )
                    rinv2 = ph3.tile([P, 1], FP32, name="rinv2", tag="rinv2")
                    nc.vector.reciprocal(rinv2, rowsum)
                    probs_f = ph3.tile([P, N], FP32, name="probs_f", tag="probs_f")
                    nc.vector.tensor_scalar_mul(probs_f, pex, rinv2[:, 0:1])
                    nc.sync.dma_start(probs_out.ap()[ic * P : (ic + 1) * P, :], probs_f)
                    nc.vector.tensor_copy(out=probs_bf[ic], in_=probs_f)

            # ---------------- Phase 4: probs transpose + context ----------------
            with (
                tc.tile_pool(name="ph4", bufs=2) as ph4,
                tc.tile_pool(name="pst2", bufs=2, space="PSUM") as pst2_pool,
                tc.tile_pool(name="psc", bufs=4, space="PSUM") as psc_pool,
            ):
                probsT = [
                    ph4.tile([P, N], BF16, name=f"probsT{jc}", tag=f"probsT{jc}", bufs=1)
                    for jc in range(2)
                ]
                for jc in range(2):
                    for ic in range(2):
                        tp2 = pst2_pool.tile([P, P], BF16, name="tp2", tag="tp2")
                        nc.tensor.transpose(
                            tp2, probs_bf[ic][:, jc * P : (jc + 1) * P], ident
                        )
                        nc.scalar.copy(probsT[jc][:, ic * P : (ic + 1) * P], tp2)
                for rc in range(R // 2):
                    cps = psc_pool.tile([P, N], FP32, name="cps", tag="cps")
                    for jc in range(2):
                        nc.tensor.matmul(
                            cps,
                            lhsT=v_T[jc][:, rc * P : (rc + 1) * P],
                            rhs=probsT[jc],
                            start=(jc == 0),
                            stop=(jc == 1),
                        )
                    cs = ph4.tile([P, N], BF16, name="cs", tag="cs", bufs=4)
                    nc.vector.tensor_copy(out=cs, in_=cps)
                    for rl in range(2):
                        nc.sync.dma_start(
                            a2a_in[2 * rc + rl],
                            cs[rl * D : (rl + 1) * D, :],
                        )

            # ---------------- Phase 5: AllToAll ----------------
            nc.gpsimd.collective_compute(
                "AllToAll",
                mybir.AluOpType.bypass,
                replica_groups=rg,
                ins=[a2a_in.opt()],
                outs=[a2a_out.opt()],
            )

            # ---------------- Phase 6: output projection ----------------
            with (
                tc.tile_pool(name="ph6", bufs=1) as ph6,
                tc.tile_pool(name="ps6", bufs=4, space="PSUM") as ps6_pool,
                tc.tile_pool(name="ph6o", bufs=4) as ph6o,
            ):
                wo_sb = ph6.tile([P, KT, E], BF16, name="wo_sb")
                nc.sync.dma_start(
                    wo_sb, wo_t.ap().rearrange("(kt p) c -> p kt c", p=P)
                )
                # gathered context, e-major: partition = (h-pair, d)
                ctx_sb = []
                for kt in range(KT):
                    t = ph6.tile([P, RS, N], BF16, name=f"ctx_sb{kt}")
                    for hh in range(2):
                        nc.sync.dma_start(
                            t[hh * D : (hh + 1) * D],
                            a2a_out[2 * kt + hh].rearrange("r d i -> d r i"),
                        )
                    ctx_sb.append(t)
                for r_loc in range(RS):
                    for ih in range(2):
                        ps6 = ps6_pool.tile([P, E], FP32, name="ps6", tag="ps6")
                        for kt in range(KT):
                            nc.tensor.matmul(
                                ps6,
                                lhsT=ctx_sb[kt][:, r_loc, ih * P : (ih + 1) * P],
                                rhs=wo_sb[:, kt],
                                start=(kt == 0),
                                stop=(kt == KT - 1),
                            )
                        osb = ph6o.tile([P, E], FP32, name="osb", tag="osb")
                        nc.vector.tensor_copy(out=osb, in_=ps6)
                        row0 = r_loc * N + ih * P
                        nc.sync.dma_start(
                            out_slice.ap()[row0 : row0 + P, :], osb
                        )

    nc.compile()
    return nc


_CACHE = {}


def _get_program():
    if "nc" not in _CACHE:
        _CACHE["nc"] = build_program()
    return _CACHE["nc"]


def _make_in_maps(x, network, Wq, bq, Wk, bk, Wv, bv, Wo, bo, Wq1, bq1, Wk1, bk1, l):
    x = np.asarray(x, np.float32)
    network = np.asarray(network, np.float32)
    for b_, nm in ((bq, "bq"), (bk, "bk"), (bv, "bv"), (bo, "bo"),
                   (bq1, "bq1"), (bk1, "bk1")):
        assert np.allclose(np.asarray(b_), 0.0), f"nonzero bias {nm} unsupported"
    wc = np.asarray(Wq1, np.float64) @ np.asarray(bk1, np.float64)
    assert np.allclose(wc, 0.0)

    # wa folds the whole q1_proj/net-k MLP into one weighted d-contraction.
    wa = (np.asarray(Wq1, np.float64) @ np.asarray(Wk1, np.float64)[0]) / H  # [D]

    xTn = np.ascontiguousarray(
        x[:, :, 0, :].transpose(2, 1, 0)
    ).astype(BF16_NP)  # [E, N, R]

    negeye = np.zeros((2, P, N), np.float32)
    for ic in range(2):
        for p in range(P):
            negeye[ic, p, ic * P + p] = NEG

    netn = np.ascontiguousarray(network[0], np.float32)  # [N, N, M]
    Wo_n = np.ascontiguousarray(np.asarray(Wo, np.float32)).astype(BF16_NP)

    in_maps = []
    for h in range(NCORES):
        sl = slice(h * D, (h + 1) * D)
        w4 = np.empty((E, 256), np.float64)
        w4[:, 0:64] = np.asarray(Wq, np.float64)[:, sl] * SCALING
        w4[:, 64:128] = np.asarray(Wk, np.float64)[:, sl]
        w4[:, 128:192] = w4[:, 0:64] * wa[None, :]
        w4[:, 192:256] = np.asarray(Wv, np.float64)[:, sl]
        in_maps.append(
            {
                "xT": xTn,
                "w4": w4.astype(BF16_NP),
                "wo_t": Wo_n,
                "lrep": np.full((P, 1), np.float32(np.asarray(l)[h, 0, 0, 0]), np.float32),
                "negeye": negeye,
                "net": netn,
            }
        )
    return in_maps


def _assemble(results, l):
    out = np.empty((R, N, B, E), np.float32)
    probs = np.empty((H, B, N, N), np.float32)
    for h in range(NCORES):
        res = results[h]
        out[h * RS : (h + 1) * RS, :, 0, :] = res["out_slice"].reshape(RS, N, E)
        probs[h, 0] = res["probs_out"]
    return out, probs, np.asarray(l, np.float32)


def kernel(**inputs):
    nc = _get_program()
    in_maps = _make_in_maps(**inputs)
    r = bass_utils.run_bass_kernel_spmd(nc, in_maps, core_ids=list(range(NCORES)))
    return _assemble(r.results, inputs["l"])


# revision 18
# speedup vs baseline: 10.0656x; 10.0656x over previous
# Trainium2 Bass kernel for nn_Attention_10342281248904 (sparse_attention).
#
# Sharding: tensor-parallel over heads H=8, one head per NeuronCore.
# Each core: q/k/v projections for its head, the hnijd Gram contraction,
# softmax + network-bias branch, context matmul. The mean-over-heads in
# q1_proj is algebraically collapsed to a single weighted d-contraction
# ("a" channel) and realized with one AllReduce; the output projection is
# done after an AllToAll that gives each core a 16-row slice of the full
# 512-channel context (row-sliced data parallel out_proj, no final
# all-reduce needed).
import math
import os
import numpy as np
import ml_dtypes

import concourse.bass as bass
import concourse.mybir as mybir
import concourse.tile as tile
from concourse import bacc
from concourse import bass_utils
from concourse.masks import make_identity

# Problem constants (hardcoded per task contract)
R, N, B, E, H, M = 128, 256, 1, 512, 8, 4
D = E // H          # 64 head dim
NCORES = 8
RS = R // NCORES    # 16 rows of R per core in the output slice
P = 128
KT = E // P         # 4 contraction tiles for E
NEG = -1.0e9
SCALING = (D ** -0.5) / math.sqrt(R)

FP32 = mybir.dt.float32
BF16 = mybir.dt.bfloat16

BF16_NP = ml_dtypes.bfloat16


def build_program(body_reps=1, phase_reps=(1, 1, 1, 1, 1, 1)):
    """Build the SPMD Bass program (same NEFF on all 8 cores; per-core
    behavior differs only through per-core input tensors).

    body_reps repeats the whole body; phase_reps[i] repeats phase i+1
    (every phase is idempotent) — both only used for slope timing."""
    nc = bacc.Bacc(
        "TRN2",
        target_bir_lowering=False,
        debug=False,
        num_devices=NCORES,
    )

    # ---- I/O ----
    xT = nc.dram_tensor("xT", [E, N, R], BF16, kind="ExternalInput")
    w4 = nc.dram_tensor("w4", [E, 256], BF16, kind="ExternalInput")
    wo_t = nc.dram_tensor("wo_t", [E, E], BF16, kind="ExternalInput")
    lrep = nc.dram_tensor("lrep", [P, 1], FP32, kind="ExternalInput")
    negeye = nc.dram_tensor("negeye", [2, P, N], FP32, kind="ExternalInput")
    net = nc.dram_tensor("net", [N, N, M], FP32, kind="ExternalInput")

    probs_out = nc.dram_tensor("probs_out", [N, N], FP32, kind="ExternalOutput")
    out_slice = nc.dram_tensor("out_slice", [RS * N, E], FP32, kind="ExternalOutput")

    xT_ap = xT.ap()
    rg = [list(range(NCORES))]

    with tile.TileContext(nc) as tc:
      for _rep in range(body_reps):
        with (
            tc.tile_pool(name="const", bufs=1) as const_pool,
            tc.tile_pool(name="big", bufs=1) as big_pool,
            tc.tile_pool(name="dram", bufs=1, space="DRAM") as dram_pool,
            tc.tile_pool(name="sm", bufs=2) as sm_pool,
        ):
            # Persistent SBUF tensors
            w4_sb = const_pool.tile([P, KT, 256], BF16, name="w4_sb")
            nc.sync.dma_start(w4_sb, w4.ap().rearrange("(kt p) c -> p kt c", p=P))
            ident = const_pool.tile([P, P], BF16, name="ident")
            make_identity(nc, ident)
            lrep_sb = const_pool.tile([P, 1], FP32, name="lrep_sb")
            nc.sync.dma_start(lrep_sb, lrep.ap())

            # DRAM bounce buffers for collectives
            ar_in = dram_pool.tile([N, N], FP32, name="ar_in")
            a2a_in = dram_pool.tile([R, D, N], BF16, name="a2a_in")

            # ---------------- Phases 1+2 share the big qkv tensor ----------------
            qkv_pool_cm = tc.tile_pool(name="qkvp", bufs=1)
            qkv_pool = qkv_pool_cm.__enter__()
            # qkv: [r, (i, ch)] with ch = [q(0:64) | k(64:128) | qa(128:192) | v(192:256)]
            qkv = qkv_pool.tile([P, N * 256], BF16, name="qkv")
            qkv_r = qkv.rearrange("p (i c) -> p i c", c=256)

            # ---------------- Phase 1: projections ----------------
            IB = 16  # i-block streamed per DMA
            for _p1 in range(phase_reps[0]):
                with (
                    tc.tile_pool(name="xt", bufs=2) as xt_pool,
                    tc.tile_pool(name="ps1", bufs=4, space="PSUM") as ps1_pool,
                ):
                    for ib in range(N // IB):
                        xts = []
                        for kt in range(KT):
                            t = xt_pool.tile(
                                [P, IB * P], BF16, name=f"xt{kt}", tag=f"xt{kt}"
                            )
                            nc.sync.dma_start(
                                t,
                                xT_ap[
                                    kt * P : (kt + 1) * P, ib * IB : (ib + 1) * IB, :
                                ].rearrange("e i r -> e (i r)"),
                            )
                            xts.append(t)
                        for ii in range(0, IB, 2):
                            ps = ps1_pool.tile([P, 512], FP32, name="ps1", tag="ps1")
                            for half in range(2):
                                i_loc = ii + half
                                for kt in range(KT):
                                    nc.tensor.matmul(
                                        ps[:, half * 256 : (half + 1) * 256],
                                        lhsT=xts[kt][:, i_loc * P : (i_loc + 1) * P],
                                        rhs=w4_sb[:, kt],
                                        start=(kt == 0),
                                        stop=(kt == KT - 1),
                                    )
                            i_glob = ib * IB + ii
                            nc.vector.tensor_copy(
                                out=qkv[:, i_glob * 256 : (i_glob + 2) * 256],
                                in_=ps,
                            )

            # ---------------- Phase 2: Gram contractions + v transpose ----------------
            # v_T[jc]: [j (128), (r, d)] bf16 for the context matmul
            v_T = [
                big_pool.tile([P, R * D], BF16, name=f"v_T{jc}") for jc in range(2)
            ]
            attn_sb = [
                sm_pool.tile([P, N], FP32, name=f"attn_sb{ic}", tag=f"attn_sb{ic}")
                for ic in range(2)
            ]
            a_sb = [
                sm_pool.tile([P, N], FP32, name=f"a_sb{ic}", tag=f"a_sb{ic}")
                for ic in range(2)
            ]

            for _p2 in range(phase_reps[1]):
                with (
                    tc.tile_pool(name="psacc", bufs=1, space="PSUM") as psacc_pool,
                    tc.tile_pool(name="pst", bufs=2, space="PSUM") as pst_pool,
                ):
                    a_ps = [
                        psacc_pool.tile([P, N], FP32, name=f"a_ps{ic}", tag=f"a_ps{ic}")
                        for ic in range(2)
                    ]
                    attn_ps = [
                        psacc_pool.tile([P, N], FP32, name=f"at_ps{ic}", tag=f"at_ps{ic}")
                        for ic in range(2)
                    ]
                    # a-channel first so the AllReduce can start early
                    for d in range(D):
                        kr = qkv_r[:, :, 64 + d]
                        for ic in range(2):
                            nc.tensor.matmul(
                                a_ps[ic],
                                lhsT=qkv_r[:, ic * P : (ic + 1) * P, 128 + d],
                                rhs=kr,
                                start=(d == 0),
                                stop=(d == D - 1),
                            )
                    for ic in range(2):
                        nc.vector.tensor_copy(out=a_sb[ic], in_=a_ps[ic])
                        nc.sync.dma_start(ar_in[ic * P : (ic + 1) * P, :], a_sb[ic])
                    ar_out = dram_pool.tile(
                        [N, N], FP32, name="ar_out", addr_space="Shared",
                        tag=f"ar_out{_p2}",
                    )
                    nc.gpsimd.collective_compute(
                        "AllReduce",
                        mybir.AluOpType.add,
                        replica_groups=rg,
                        ins=[ar_in.opt()],
                        outs=[ar_out.opt()],
                    )

                    # attn channel
                    for d in range(D):
                        kr = qkv_r[:, :, 64 + d]
                        for ic in range(2):
                            nc.tensor.matmul(
                                attn_ps[ic],
                                lhsT=qkv_r[:, ic * P : (ic + 1) * P, d],
                                rhs=kr,
                                start=(d == 0),
                                stop=(d == D - 1),
                            )
                    for ic in range(2):
                        nc.vector.tensor_copy(out=attn_sb[ic], in_=attn_ps[ic])

                    # v transposes: qkv [r, (j, 192+d)] -> v_T[jc] [j, (r, d)]
                    for jc in range(2):
                        vtr = v_T[jc].rearrange("p (r d) -> p r d", d=D)
                        for d in range(D):
                            tp = pst_pool.tile([P, P], BF16, name="tp", tag="tp")
                            nc.tensor.transpose(
                                tp, qkv_r[:, jc * P : (jc + 1) * P, 192 + d], ident
                            )
                            nc.scalar.copy(vtr[:, :, d], tp)
            qkv_pool_cm.__exit__(None, None, None)

            # ---------------- Phase 3: network branch + softmax ----------------
            probs_bf = [
                big_pool.tile([P, N], BF16, name=f"probs_bf{ic}") for ic in range(2)
            ]
            for _p3 in range(phase_reps[2]):
                with tc.tile_pool(name="ph3", bufs=1) as ph3:
                    a_full = [
                        ph3.tile([P, N], FP32, name=f"a_full{ic}", tag=f"a_full{ic}")
                        for ic in range(2)
                    ]
                    for ic in range(2):
                        nc.sync.dma_start(a_full[ic], ar_out[ic * P : (ic + 1) * P, :])
                    for ic in range(2):
                        net_sb = ph3.tile([P, N * M], FP32, name="net_sb", tag="net_sb")
                        nc.sync.dma_start(
                            net_sb,
                            net.ap()[ic * P : (ic + 1) * P].rearrange("i j m -> i (j m)"),
                        )
                        net_r = net_sb.rearrange("p (j m) -> p j m", m=M)
                        na = ph3.tile([P, N * M], FP32, name="na", tag="na")
                        na_r = na.rearrange("p (j m) -> p j m", m=M)
                        a_b = a_full[ic][:, :, None].to_broadcast([P, N, M])
                        nc.vector.tensor_tensor(na_r, net_r, a_b, mybir.AluOpType.mult)
                        # softmax over m (4 channels); |na| << 1 so no max-shift needed
                        ne = ph3.tile([P, N * M], FP32, name="ne", tag="ne")
                        nc.scalar.activation(
                            ne, na, mybir.ActivationFunctionType.Exp
                        )
                        ne_r = ne.rearrange("p (j m) -> p j m", m=M)
                        s4 = ph3.tile([P, N], FP32, name="s4", tag="s4")
                        nc.vector.reduce_sum(s4, ne_r, axis=mybir.AxisListType.X)
                        rinv = ph3.tile([P, N], FP32, name="rinv", tag="rinv")
                        nc.vector.reciprocal(rinv, s4)
                        # net_bias = (sum_m network * exp) / sum_m exp
                        tw = ph3.tile([P, N * M], FP32, name="tw", tag="tw")
                        nc.vector.tensor_tensor(tw, ne, net_sb, mybir.AluOpType.mult)
                        tw_r = tw.rearrange("p (j m) -> p j m", m=M)
                        nb = ph3.tile([P, N], FP32, name="nb", tag="nb")
                        nc.vector.reduce_sum(nb, tw_r, axis=mybir.AxisListType.X)
                        nc.vector.tensor_tensor(nb, nb, rinv, mybir.AluOpType.mult)
                        # attn_final = attn + l * net_bias + negeye
                        lnb = ph3.tile([P, N], FP32, name="lnb", tag="lnb")
                        nc.vector.tensor_scalar_mul(lnb, nb, lrep_sb[:, 0:1])
                        negeye_sb = ph3.tile([P, N], FP32, name="negeye_sb", tag="ney")
                        nc.sync.dma_start(negeye_sb, negeye.ap()[ic])
                        af = ph3.tile([P, N], FP32, name="af", tag="af")
                        nc.vector.tensor_tensor(af, attn_sb[ic], lnb, mybir.AluOpType.add)
                        nc.vector.tensor_tensor(af, af, negeye_sb, mybir.AluOpType.add)
                        # row softmax over j
                        negmx = ph3.tile([P, 1], FP32, name="negmx", tag="negmx")
                        nc.vector.tensor_reduce(
                            negmx, af, axis=mybir.AxisListType.X,
                            op=mybir.AluOpType.max, negate=True,
                        )
                        pex = ph3.tile([P, N], FP32, name="pex", tag="pex")
                        rowsum = ph3.tile([P, 1], FP32, name="rowsum", tag="rowsum")
                        nc.scalar.activation(
                            pex, af, mybir.ActivationFunctionType.Exp,
                            bias=negmx[:, 0:1], accum_out=rowsum[:, 0:1],
                        )
                        rinv2 = ph3.tile([P, 1], FP32, name="rinv2", tag="rinv2")
                        nc.vector.reciprocal(rinv2, rowsum)
                        probs_f = ph3.tile([P, N], FP32, name="probs_f", tag="probs_f")
                        nc.vector.tensor_scalar_mul(probs_f, pex, rinv2[:, 0:1])
                        nc.sync.dma_start(
                            probs_out.ap()[ic * P : (ic + 1) * P, :], probs_f
                        )
                        nc.vector.tensor_copy(out=probs_bf[ic], in_=probs_f)

            # ---------------- Phase 4: probs transpose + context ----------------
            for _p4 in range(phase_reps[3]):
                with (
                    tc.tile_pool(name="ph4", bufs=2) as ph4,
                    tc.tile_pool(name="pst2", bufs=2, space="PSUM") as pst2_pool,
                    tc.tile_pool(name="psc", bufs=4, space="PSUM") as psc_pool,
                ):
                    probsT = [
                        ph4.tile([P, N], BF16, name=f"probsT{jc}", tag=f"probsT{jc}",
                                 bufs=1)
                        for jc in range(2)
                    ]
                    for jc in range(2):
                        for ic in range(2):
                            tp2 = pst2_pool.tile([P, P], BF16, name="tp2", tag="tp2")
                            nc.tensor.transpose(
                                tp2, probs_bf[ic][:, jc * P : (jc + 1) * P], ident
                            )
                            nc.scalar.copy(probsT[jc][:, ic * P : (ic + 1) * P], tp2)
                    for rc in range(R // 2):
                        cps = psc_pool.tile([P, N], FP32, name="cps", tag="cps")
                        for jc in range(2):
                            nc.tensor.matmul(
                                cps,
                                lhsT=v_T[jc][:, rc * P : (rc + 1) * P],
                                rhs=probsT[jc],
                                start=(jc == 0),
                                stop=(jc == 1),
                            )
                        cs = ph4.tile([P, N], BF16, name="cs", tag="cs", bufs=4)
                        nc.vector.tensor_copy(out=cs, in_=cps)
                        for rl in range(2):
                            nc.sync.dma_start(
                                a2a_in[2 * rc + rl],
                                cs[rl * D : (rl + 1) * D, :],
                            )

            # ---------------- Phase 5: AllToAll ----------------
            for _p5 in range(phase_reps[4]):
                a2a_out = dram_pool.tile(
                    [NCORES, RS, D, N], BF16, name="a2a_out", tag=f"a2a_out{_p5}"
                )
                nc.gpsimd.collective_compute(
                    "AllToAll",
                    mybir.AluOpType.bypass,
                    replica_groups=rg,
                    ins=[a2a_in.opt()],
                    outs=[a2a_out.opt()],
                )

            # ---------------- Phase 6: output projection ----------------
            for _p6 in range(phase_reps[5]):
                with (
                    tc.tile_pool(name="ph6", bufs=1) as ph6,
                    tc.tile_pool(name="ps6", bufs=4, space="PSUM") as ps6_pool,
                    tc.tile_pool(name="ph6o", bufs=4) as ph6o,
                ):
                    wo_sb = ph6.tile([P, KT, E], BF16, name="wo_sb")
                    nc.sync.dma_start(
                        wo_sb, wo_t.ap().rearrange("(kt p) c -> p kt c", p=P)
                    )
                    # gathered context, e-major: partition = (h-pair, d)
                    ctx_sb = []
                    for kt in range(KT):
                        t = ph6.tile([P, RS, N], BF16, name=f"ctx_sb{kt}")
                        for hh in range(2):
                            nc.sync.dma_start(
                                t[hh * D : (hh + 1) * D],
                                a2a_out[2 * kt + hh].rearrange("r d i -> d r i"),
                            )
                        ctx_sb.append(t)
                    for r_loc in range(RS):
                        for ih in range(2):
                            ps6 = ps6_pool.tile([P, E], FP32, name="ps6", tag="ps6")
                            for kt in range(KT):
                                nc.tensor.matmul(
                                    ps6,
                                    lhsT=ctx_sb[kt][:, r_loc, ih * P : (ih + 1) * P],
                                    rhs=wo_sb[:, kt],
                                    start=(kt == 0),
                                    stop=(kt == KT - 1),
                                )
                            osb = ph6o.tile([P, E], FP32, name="osb", tag="osb")
                            nc.vector.tensor_copy(out=osb, in_=ps6)
                            row0 = r_loc * N + ih * P
                            nc.sync.dma_start(
                                out_slice.ap()[row0 : row0 + P, :], osb
                            )

    nc.compile()
    return nc


_CACHE = {}


def _get_program():
    if "nc" not in _CACHE:
        _CACHE["nc"] = build_program()
    return _CACHE["nc"]


def _make_in_maps(x, network, Wq, bq, Wk, bk, Wv, bv, Wo, bo, Wq1, bq1, Wk1, bk1, l):
    x = np.asarray(x, np.float32)
    network = np.asarray(network, np.float32)
    for b_, nm in ((bq, "bq"), (bk, "bk"), (bv, "bv"), (bo, "bo"),
                   (bq1, "bq1"), (bk1, "bk1")):
        assert np.allclose(np.asarray(b_), 0.0), f"nonzero bias {nm} unsupported"
    wc = np.asarray(Wq1, np.float64) @ np.asarray(bk1, np.float64)
    assert np.allclose(wc, 0.0)

    # wa folds the whole q1_proj/net-k MLP into one weighted d-contraction.
    wa = (np.asarray(Wq1, np.float64) @ np.asarray(Wk1, np.float64)[0]) / H  # [D]

    xTn = np.ascontiguousarray(
        x[:, :, 0, :].transpose(2, 1, 0)
    ).astype(BF16_NP)  # [E, N, R]

    negeye = np.zeros((2, P, N), np.float32)
    for ic in range(2):
        for p in range(P):
            negeye[ic, p, ic * P + p] = NEG

    netn = np.ascontiguousarray(network[0], np.float32)  # [N, N, M]
    Wo_n = np.ascontiguousarray(np.asarray(Wo, np.float32)).astype(BF16_NP)

    in_maps = []
    for h in range(NCORES):
        sl = slice(h * D, (h + 1) * D)
        w4v = np.empty((E, 256), np.float64)
        w4v[:, 0:64] = np.asarray(Wq, np.float64)[:, sl] * SCALING
        w4v[:, 64:128] = np.asarray(Wk, np.float64)[:, sl]
        w4v[:, 128:192] = w4v[:, 0:64] * wa[None, :]
        w4v[:, 192:256] = np.asarray(Wv, np.float64)[:, sl]
        in_maps.append(
            {
                "xT": xTn,
                "w4": w4v.astype(BF16_NP),
                "wo_t": Wo_n,
                "lrep": np.full((P, 1), np.float32(np.asarray(l)[h, 0, 0, 0]),
                                np.float32),
                "negeye": negeye,
                "net": netn,
            }
        )
    return in_maps


def _assemble(results, l):
    out = np.empty((R, N, B, E), np.float32)
    probs = np.empty((H, B, N, N), np.float32)
    for h in range(NCORES):
        res = results[h]
        out[h * RS : (h + 1) * RS, :, 0, :] = res["out_slice"].reshape(RS, N, E)
        probs[h, 0] = res["probs_out"]
    return out, probs, np.asarray(l, np.float32)


def kernel(**inputs):
    nc = _get_program()
    in_maps = _make_in_maps(**inputs)
    r = bass_utils.run_bass_kernel_spmd(nc, in_maps, core_ids=list(range(NCORES)))
    return _assemble(r.results, inputs["l"])


# revision 20
# speedup vs baseline: 11.0035x; 1.0932x over previous
# Trainium2 Bass kernel for nn_Attention_10342281248904 (sparse_attention).
#
# Sharding: tensor-parallel over heads H=8, one head per NeuronCore.
# Each core: q/k/v projections for its head, the hnijd Gram contraction,
# softmax + network-bias branch, context matmul. The mean-over-heads in
# q1_proj is algebraically collapsed to a single weighted d-contraction
# ("a" channel) and realized with one AllReduce; the output projection is
# done after an AllToAll that gives each core a 16-row slice of the full
# 512-channel context (row-sliced data parallel out_proj, no final
# all-reduce needed).
import math
import os
import numpy as np
import ml_dtypes

import concourse.bass as bass
import concourse.mybir as mybir
import concourse.tile as tile
from concourse import bacc
from concourse import bass_utils
from concourse.masks import make_identity

# Problem constants (hardcoded per task contract)
R, N, B, E, H, M = 128, 256, 1, 512, 8, 4
D = E // H          # 64 head dim
NCORES = 8
RS = R // NCORES    # 16 rows of R per core in the output slice
P = 128
KT = E // P         # 4 contraction tiles for E
NEG = -1.0e9
SCALING = (D ** -0.5) / math.sqrt(R)

FP32 = mybir.dt.float32
BF16 = mybir.dt.bfloat16

BF16_NP = ml_dtypes.bfloat16


def build_program(body_reps=1, phase_reps=(1, 1, 1, 1, 1, 1)):
    """Build the SPMD Bass program (same NEFF on all 8 cores; per-core
    behavior differs only through per-core input tensors).

    body_reps repeats the whole body; phase_reps[i] repeats phase i+1
    (every phase is idempotent) — both only used for slope timing."""
    nc = bacc.Bacc(
        "TRN2",
        target_bir_lowering=False,
        debug=False,
        num_devices=NCORES,
    )

    # ---- I/O ----
    xT = nc.dram_tensor("xT", [E, N, R], BF16, kind="ExternalInput")
    w4 = nc.dram_tensor("w4", [E, 256], BF16, kind="ExternalInput")
    wo_t = nc.dram_tensor("wo_t", [E, E], BF16, kind="ExternalInput")
    lrep = nc.dram_tensor("lrep", [P, 1], FP32, kind="ExternalInput")
    negeye = nc.dram_tensor("negeye", [2, P, N], FP32, kind="ExternalInput")
    net = nc.dram_tensor("net", [N, N, M], FP32, kind="ExternalInput")

    probs_out = nc.dram_tensor("probs_out", [N, N], FP32, kind="ExternalOutput")
    out_slice = nc.dram_tensor("out_slice", [RS * N, E], FP32, kind="ExternalOutput")

    xT_ap = xT.ap()
    rg = [list(range(NCORES))]

    with tile.TileContext(nc) as tc:
      for _rep in range(body_reps):
        with (
            tc.tile_pool(name="const", bufs=1) as const_pool,
            tc.tile_pool(name="big", bufs=1) as big_pool,
            tc.tile_pool(name="dram", bufs=1, space="DRAM") as dram_pool,
            tc.tile_pool(name="sm", bufs=2) as sm_pool,
        ):
            # Persistent SBUF tensors
            w4_sb = const_pool.tile([P, KT, 256], BF16, name="w4_sb")
            nc.sync.dma_start(w4_sb, w4.ap().rearrange("(kt p) c -> p kt c", p=P))
            ident = const_pool.tile([P, P], BF16, name="ident")
            make_identity(nc, ident)
            lrep_sb = const_pool.tile([P, 1], FP32, name="lrep_sb")
            nc.sync.dma_start(lrep_sb, lrep.ap())
            wo_sb = const_pool.tile([P, KT, E], BF16, name="wo_sb")
            nc.sync.dma_start(
                wo_sb, wo_t.ap().rearrange("(kt p) c -> p kt c", p=P)
            )

            # DRAM bounce buffers for collectives
            ar_in = dram_pool.tile([N, N], FP32, name="ar_in")
            a2a_in = dram_pool.tile([R, D, N], BF16, name="a2a_in")

            # ---------------- Phases 1+2 share the big qkv tensors ----------------
            qkv_pool_cm = tc.tile_pool(name="qkvp", bufs=1)
            qkv_pool = qkv_pool_cm.__enter__()
            # qkv3: [r, (i, ch)] with ch = [q(0:64) | qa(64:128) | v(128:192)]
            # k_dm: [r, (d, j)]  d-major so the Gram moving operand is contiguous
            qkv3 = qkv_pool.tile([P, N * 192], BF16, name="qkv3")
            qkv_r = qkv3.rearrange("p (i c) -> p i c", c=192)
            k_dm = qkv_pool.tile([P, D * N], BF16, name="k_dm")
            k_dm_r = k_dm.rearrange("p (d j) -> p d j", j=N)

            # ---------------- Phase 1: projections ----------------
            IB = 16  # i-block streamed per DMA
            for _p1 in range(phase_reps[0]):
                with (
                    tc.tile_pool(name="xt", bufs=2) as xt_pool,
                    tc.tile_pool(name="ps1", bufs=3, space="PSUM") as ps1_pool,
                ):
                    for ib in range(N // IB):
                        xts = []
                        for kt in range(KT):
                            t = xt_pool.tile(
                                [P, IB * P], BF16, name=f"xt{kt}", tag=f"xt{kt}"
                            )
                            nc.sync.dma_start(
                                t,
                                xT_ap[
                                    kt * P : (kt + 1) * P, ib * IB : (ib + 1) * IB, :
                                ].rearrange("e i r -> e (i r)"),
                            )
                            xts.append(t)
                        for ii in range(0, IB, 4):
                            ps = ps1_pool.tile([P, 1024], FP32, name="ps1", tag="ps1")
                            ps_r = ps.rearrange("p (i c) -> p i c", c=256)
                            for quar in range(4):
                                i_loc = ii + quar
                                for kt in range(KT):
                                    nc.tensor.matmul(
                                        ps[:, quar * 256 : (quar + 1) * 256],
                                        lhsT=xts[kt][:, i_loc * P : (i_loc + 1) * P],
                                        rhs=w4_sb[:, kt],
                                        start=(kt == 0),
                                        stop=(kt == KT - 1),
                                    )
                            i_glob = ib * IB + ii
                            nc.vector.tensor_copy(
                                out=qkv_r[:, i_glob : i_glob + 4, :],
                                in_=ps_r[:, :, 0:192],
                            )
                            nc.vector.tensor_copy(
                                out=k_dm_r[:, :, i_glob : i_glob + 4].rearrange(
                                    "p d i -> p i d"
                                ),
                                in_=ps_r[:, :, 192:256],
                            )

            # ---------------- Phase 2: Gram contractions + v transpose ----------------
            # v_T[jc]: [j (128), (r, d)] bf16 for the context matmul
            v_T = [
                big_pool.tile([P, R * D], BF16, name=f"v_T{jc}") for jc in range(2)
            ]
            attn_sb = [
                sm_pool.tile([P, N], FP32, name=f"attn_sb{ic}", tag=f"attn_sb{ic}")
                for ic in range(2)
            ]
            a_sb = [
                sm_pool.tile([P, N], FP32, name=f"a_sb{ic}", tag=f"a_sb{ic}")
                for ic in range(2)
            ]

            for _p2 in range(phase_reps[1]):
                with (
                    tc.tile_pool(name="psacc", bufs=1, space="PSUM") as psacc_pool,
                    tc.tile_pool(name="pst", bufs=3, space="PSUM") as pst_pool,
                ):
                    a_ps = [
                        psacc_pool.tile([P, N], FP32, name=f"a_ps{ic}", tag=f"a_ps{ic}")
                        for ic in range(2)
                    ]
                    attn_ps = [
                        psacc_pool.tile([P, N], FP32, name=f"at_ps{ic}", tag=f"at_ps{ic}")
                        for ic in range(2)
                    ]
                    # a-channel first so the AllReduce can start early
                    for d in range(D):
                        kr = k_dm_r[:, d]
                        for ic in range(2):
                            nc.tensor.matmul(
                                a_ps[ic],
                                lhsT=qkv_r[:, ic * P : (ic + 1) * P, 64 + d],
                                rhs=kr,
                                start=(d == 0),
                                stop=(d == D - 1),
                            )
                    for ic in range(2):
                        nc.vector.tensor_copy(out=a_sb[ic], in_=a_ps[ic])
                        nc.sync.dma_start(ar_in[ic * P : (ic + 1) * P, :], a_sb[ic])
                    ar_out = dram_pool.tile(
                        [N, N], FP32, name="ar_out", addr_space="Shared",
                        tag=f"ar_out{_p2}",
                    )
                    nc.gpsimd.collective_compute(
                        "AllReduce",
                        mybir.AluOpType.add,
                        replica_groups=rg,
                        ins=[ar_in.opt()],
                        outs=[ar_out.opt()],
                    )

                    # attn channel
                    for d in range(D):
                        kr = k_dm_r[:, d]
                        for ic in range(2):
                            nc.tensor.matmul(
                                attn_ps[ic],
                                lhsT=qkv_r[:, ic * P : (ic + 1) * P, d],
                                rhs=kr,
                                start=(d == 0),
                                stop=(d == D - 1),
                            )
                    for ic in range(2):
                        nc.vector.tensor_copy(out=attn_sb[ic], in_=attn_ps[ic])

                    # v transposes: qkv [r, (j, 128+d)] -> v_T[jc] [j, (r, d)]
                    for jc in range(2):
                        vtr = v_T[jc].rearrange("p (r d) -> p r d", d=D)
                        for d in range(D):
                            tp = pst_pool.tile([P, P], BF16, name="tp", tag="tp")
                            nc.tensor.transpose(
                                tp, qkv_r[:, jc * P : (jc + 1) * P, 128 + d], ident
                            )
                            nc.vector.tensor_copy(out=vtr[:, :, d], in_=tp)
            qkv_pool_cm.__exit__(None, None, None)

            # ---------------- Phase 3: network branch + softmax ----------------
            probs_bf = [
                big_pool.tile([P, N], BF16, name=f"probs_bf{ic}") for ic in range(2)
            ]
            for _p3 in range(phase_reps[2]):
                with tc.tile_pool(name="ph3", bufs=1) as ph3:
                    a_full = [
                        ph3.tile([P, N], FP32, name=f"a_full{ic}", tag=f"a_full{ic}")
                        for ic in range(2)
                    ]
                    for ic in range(2):
                        nc.sync.dma_start(a_full[ic], ar_out[ic * P : (ic + 1) * P, :])
                    for ic in range(2):
                        net_sb = ph3.tile([P, N * M], FP32, name="net_sb", tag="net_sb")
                        nc.sync.dma_start(
                            net_sb,
                            net.ap()[ic * P : (ic + 1) * P].rearrange("i j m -> i (j m)"),
                        )
                        net_r = net_sb.rearrange("p (j m) -> p j m", m=M)
                        na = ph3.tile([P, N * M], FP32, name="na", tag="na")
                        na_r = na.rearrange("p (j m) -> p j m", m=M)
                        a_b = a_full[ic][:, :, None].to_broadcast([P, N, M])
                        nc.vector.tensor_tensor(na_r, net_r, a_b, mybir.AluOpType.mult)
                        # softmax over m (4 channels); |na| << 1 so no max-shift needed
                        ne = ph3.tile([P, N * M], FP32, name="ne", tag="ne")
                        nc.scalar.activation(
                            ne, na, mybir.ActivationFunctionType.Exp
                        )
                        ne_r = ne.rearrange("p (j m) -> p j m", m=M)
                        s4 = ph3.tile([P, N], FP32, name="s4", tag="s4")
                        nc.vector.reduce_sum(s4, ne_r, axis=mybir.AxisListType.X)
                        rinv = ph3.tile([P, N], FP32, name="rinv", tag="rinv")
                        nc.vector.reciprocal(rinv, s4)
                        # net_bias = (sum_m network * exp) / sum_m exp
                        tw = ph3.tile([P, N * M], FP32, name="tw", tag="tw")
                        nc.vector.tensor_tensor(tw, ne, net_sb, mybir.AluOpType.mult)
                        tw_r = tw.rearrange("p (j m) -> p j m", m=M)
                        nb = ph3.tile([P, N], FP32, name="nb", tag="nb")
                        nc.vector.reduce_sum(nb, tw_r, axis=mybir.AxisListType.X)
                        nc.vector.tensor_tensor(nb, nb, rinv, mybir.AluOpType.mult)
                        # attn_final = attn + l * net_bias + negeye
                        lnb = ph3.tile([P, N], FP32, name="lnb", tag="lnb")
                        nc.vector.tensor_scalar_mul(lnb, nb, lrep_sb[:, 0:1])
                        negeye_sb = ph3.tile([P, N], FP32, name="negeye_sb", tag="ney")
                        nc.sync.dma_start(negeye_sb, negeye.ap()[ic])
                        af = ph3.tile([P, N], FP32, name="af", tag="af")
                        nc.vector.tensor_tensor(af, attn_sb[ic], lnb, mybir.AluOpType.add)
                        nc.vector.tensor_tensor(af, af, negeye_sb, mybir.AluOpType.add)
                        # row softmax over j
                        negmx = ph3.tile([P, 1], FP32, name="negmx", tag="negmx")
                        nc.vector.tensor_reduce(
                            negmx, af, axis=mybir.AxisListType.X,
                            op=mybir.AluOpType.max, negate=True,
                        )
                        pex = ph3.tile([P, N], FP32, name="pex", tag="pex")
                        rowsum = ph3.tile([P, 1], FP32, name="rowsum", tag="rowsum")
                        nc.scalar.activation(
                            pex, af, mybir.ActivationFunctionType.Exp,
                            bias=negmx[:, 0:1], accum_out=rowsum[:, 0:1],
                        )
                        rinv2 = ph3.tile([P, 1], FP32, name="rinv2", tag="rinv2")
                        nc.vector.reciprocal(rinv2, rowsum)
                        probs_f = ph3.tile([P, N], FP32, name="probs_f", tag="probs_f")
                        nc.vector.tensor_scalar_mul(probs_f, pex, rinv2[:, 0:1])
                        nc.sync.dma_start(
                            probs_out.ap()[ic * P : (ic + 1) * P, :], probs_f
                        )
                        nc.vector.tensor_copy(out=probs_bf[ic], in_=probs_f)

            # ---------------- Phase 4: probs transpose + context ----------------
            for _p4 in range(phase_reps[3]):
                with (
                    tc.tile_pool(name="ph4", bufs=2) as ph4,
                    tc.tile_pool(name="pst2", bufs=2, space="PSUM") as pst2_pool,
                    tc.tile_pool(name="psc", bufs=6, space="PSUM") as psc_pool,
                ):
                    probsT = [
                        ph4.tile([P, N], BF16, name=f"probsT{jc}", tag=f"probsT{jc}",
                                 bufs=1)
                        for jc in range(2)
                    ]
                    for jc in range(2):
                        for ic in range(2):
                            tp2 = pst2_pool.tile([P, P], BF16, name="tp2", tag="tp2")
                            nc.tensor.transpose(
                                tp2, probs_bf[ic][:, jc * P : (jc + 1) * P], ident
                            )
                            nc.vector.tensor_copy(
                                out=probsT[jc][:, ic * P : (ic + 1) * P], in_=tp2
                            )
                    for rc in range(R // 2):
                        cps = psc_pool.tile([P, N], FP32, name="cps", tag="cps")
                        for jc in range(2):
                            nc.tensor.matmul(
                                cps,
                                lhsT=v_T[jc][:, rc * P : (rc + 1) * P],
                                rhs=probsT[jc],
                                start=(jc == 0),
                                stop=(jc == 1),
                            )
                        cs = ph4.tile([P, N], BF16, name="cs", tag="cs", bufs=4)
                        nc.vector.tensor_copy(out=cs, in_=cps)
                        for rl in range(2):
                            nc.sync.dma_start(
                                a2a_in[2 * rc + rl],
                                cs[rl * D : (rl + 1) * D, :],
                            )

            # ---------------- Phase 5: AllToAll ----------------
            for _p5 in range(phase_reps[4]):
                a2a_out = dram_pool.tile(
                    [NCORES, RS, D, N], BF16, name="a2a_out", tag=f"a2a_out{_p5}"
                )
                nc.gpsimd.collective_compute(
                    "AllToAll",
                    mybir.AluOpType.bypass,
                    replica_groups=rg,
                    ins=[a2a_in.opt()],
                    outs=[a2a_out.opt()],
                )

            # ---------------- Phase 6: output projection ----------------
            for _p6 in range(phase_reps[5]):
                with (
                    tc.tile_pool(name="ph6", bufs=1) as ph6,
                    tc.tile_pool(name="ps6", bufs=4, space="PSUM") as ps6_pool,
                    tc.tile_pool(name="ph6o", bufs=4) as ph6o,
                ):
                    # gathered context, e-major: partition = (h-pair, d)
                    ctx_sb = []
                    for kt in range(KT):
                        t = ph6.tile([P, RS, N], BF16, name=f"ctx_sb{kt}")
                        for hh in range(2):
                            for rh in range(2):
                                nc.sync.dma_start(
                                    t[hh * D : (hh + 1) * D, rh * 8 : (rh + 1) * 8],
                                    a2a_out[2 * kt + hh][
                                        rh * 8 : (rh + 1) * 8
                                    ].rearrange("r d i -> d r i"),
                                )
                        ctx_sb.append(t)
                    for r_loc in range(RS):
                        for ih in range(2):
                            ps6 = ps6_pool.tile([P, E], FP32, name="ps6", tag="ps6")
                            for kt in range(KT):
                                nc.tensor.matmul(
                                    ps6,
                                    lhsT=ctx_sb[kt][:, r_loc, ih * P : (ih + 1) * P],
                                    rhs=wo_sb[:, kt],
                                    start=(kt == 0),
                                    stop=(kt == KT - 1),
                                )
                            osb = ph6o.tile([P, E], FP32, name="osb", tag="osb")
                            nc.vector.tensor_copy(out=osb, in_=ps6)
                            row0 = r_loc * N + ih * P
                            nc.sync.dma_start(
                                out_slice.ap()[row0 : row0 + P, :], osb
                            )

    nc.compile()
    return nc


_CACHE = {}


def _get_program():
    if "nc" not in _CACHE:
        _CACHE["nc"] = build_program()
    return _CACHE["nc"]


def _make_in_maps(x, network, Wq, bq, Wk, bk, Wv, bv, Wo, bo, Wq1, bq1, Wk1, bk1, l):
    x = np.asarray(x, np.float32)
    network = np.asarray(network, np.float32)
    for b_, nm in ((bq, "bq"), (bk, "bk"), (bv, "bv"), (bo, "bo"),
                   (bq1, "bq1"), (bk1, "bk1")):
        assert np.allclose(np.asarray(b_), 0.0), f"nonzero bias {nm} unsupported"
    wc = np.asarray(Wq1, np.float64) @ np.asarray(bk1, np.float64)
    assert np.allclose(wc, 0.0)

    # wa folds the whole q1_proj/net-k MLP into one weighted d-contraction.
    wa = (np.asarray(Wq1, np.float64) @ np.asarray(Wk1, np.float64)[0]) / H  # [D]

    xTn = np.ascontiguousarray(
        x[:, :, 0, :].transpose(2, 1, 0)
    ).astype(BF16_NP)  # [E, N, R]

    negeye = np.zeros((2, P, N), np.float32)
    for ic in range(2):
        for p in range(P):
            negeye[ic, p, ic * P + p] = NEG

    netn = np.ascontiguousarray(network[0], np.float32)  # [N, N, M]
    Wo_n = np.ascontiguousarray(np.asarray(Wo, np.float32)).astype(BF16_NP)

    in_maps = []
    for h in range(NCORES):
        sl = slice(h * D, (h + 1) * D)
        # column order must match the kernel: [q | qa | v | k]
        w4v = np.empty((E, 256), np.float64)
        w4v[:, 0:64] = np.asarray(Wq, np.float64)[:, sl] * SCALING
        w4v[:, 64:128] = w4v[:, 0:64] * wa[None, :]
        w4v[:, 128:192] = np.asarray(Wv, np.float64)[:, sl]
        w4v[:, 192:256] = np.asarray(Wk, np.float64)[:, sl]
        in_maps.append(
            {
                "xT": xTn,
                "w4": w4v.astype(BF16_NP),
                "wo_t": Wo_n,
                "lrep": np.full((P, 1), np.float32(np.asarray(l)[h, 0, 0, 0]),
                                np.float32),
                "negeye": negeye,
                "net": netn,
            }
        )
    return in_maps


def _assemble(results, l):
    out = np.empty((R, N, B, E), np.float32)
    probs = np.empty((H, B, N, N), np.float32)
    for h in range(NCORES):
        res = results[h]
        out[h * RS : (h + 1) * RS, :, 0, :] = res["out_slice"].reshape(RS, N, E)
        probs[h, 0] = res["probs_out"]
    return out, probs, np.asarray(l, np.float32)


def kernel(**inputs):
    nc = _get_program()
    in_maps = _make_in_maps(**inputs)
    r = bass_utils.run_bass_kernel_spmd(nc, in_maps, core_ids=list(range(NCORES)))
    return _assemble(r.results, inputs["l"])


# revision 21
# speedup vs baseline: 13.6135x; 1.2372x over previous
# Trainium2 Bass kernel for nn_Attention_10342281248904 (sparse_attention).
#
# Sharding: tensor-parallel over heads H=8, one head per NeuronCore.
# Each core: q/k/v projections for its head, the hnijd Gram contraction,
# softmax + network-bias branch, context matmul. The mean-over-heads in
# q1_proj is algebraically collapsed to a single weighted d-contraction
# ("a" channel) and realized with one AllReduce; the output projection is
# done after an AllToAll that gives each core a 16-row slice of the full
# 512-channel context (row-sliced data parallel out_proj, no final
# all-reduce needed).
import math
import os
import numpy as np
import ml_dtypes

import concourse.bass as bass
import concourse.mybir as mybir
import concourse.tile as tile
from concourse import bacc
from concourse import bass_utils
from concourse.masks import make_identity

# Problem constants (hardcoded per task contract)
R, N, B, E, H, M = 128, 256, 1, 512, 8, 4
D = E // H          # 64 head dim
NCORES = 8
RS = R // NCORES    # 16 rows of R per core in the output slice
P = 128
KT = E // P         # 4 contraction tiles for E
NEG = -1.0e9
SCALING = (D ** -0.5) / math.sqrt(R)

FP32 = mybir.dt.float32
BF16 = mybir.dt.bfloat16

BF16_NP = ml_dtypes.bfloat16


def build_program(body_reps=1, phase_reps=(1, 1, 1, 1, 1, 1)):
    """Build the SPMD Bass program (same NEFF on all 8 cores; per-core
    behavior differs only through per-core input tensors).

    body_reps repeats the whole body; phase_reps[i] repeats phase i+1
    (every phase is idempotent) — both only used for slope timing."""
    nc = bacc.Bacc(
        "TRN2",
        target_bir_lowering=False,
        debug=False,
        num_devices=NCORES,
    )

    # ---- I/O ----
    xT = nc.dram_tensor("xT", [E, N, R], BF16, kind="ExternalInput")
    w4 = nc.dram_tensor("w4", [E, 256], BF16, kind="ExternalInput")
    wo_t = nc.dram_tensor("wo_t", [E, E], BF16, kind="ExternalInput")
    lrep = nc.dram_tensor("lrep", [P, 1], FP32, kind="ExternalInput")
    negeye = nc.dram_tensor("negeye", [2, P, N], FP32, kind="ExternalInput")
    net = nc.dram_tensor("net", [N, N, M], FP32, kind="ExternalInput")

    probs_out = nc.dram_tensor("probs_out", [N, N], FP32, kind="ExternalOutput")
    out_slice = nc.dram_tensor("out_slice", [RS * N, E], FP32, kind="ExternalOutput")

    xT_ap = xT.ap()
    rg = [list(range(NCORES))]

    with tile.TileContext(nc) as tc:
      for _rep in range(body_reps):
        with (
            tc.tile_pool(name="const", bufs=1) as const_pool,
            tc.tile_pool(name="big", bufs=1) as big_pool,
            tc.tile_pool(name="dram", bufs=1, space="DRAM") as dram_pool,
            tc.tile_pool(name="sm", bufs=2) as sm_pool,
        ):
            # Persistent SBUF tensors
            w4_sb = const_pool.tile([P, KT, 256], BF16, name="w4_sb")
            nc.sync.dma_start(w4_sb, w4.ap().rearrange("(kt p) c -> p kt c", p=P))
            ident = const_pool.tile([P, P], BF16, name="ident")
            make_identity(nc, ident)
            lrep_sb = const_pool.tile([P, 1], FP32, name="lrep_sb")
            nc.sync.dma_start(lrep_sb, lrep.ap())
            wo_sb = const_pool.tile([P, KT, E], BF16, name="wo_sb")
            nc.sync.dma_start(
                wo_sb, wo_t.ap().rearrange("(kt p) c -> p kt c", p=P)
            )

            # DRAM bounce buffers for collectives
            ar_in = dram_pool.tile([N, N], FP32, name="ar_in")
            a2a_in = dram_pool.tile([R, D, N], BF16, name="a2a_in")

            # ---------------- Phases 1+2 share the big qkv tensors ----------------
            qkv_pool_cm = tc.tile_pool(name="qkvp", bufs=1)
            qkv_pool = qkv_pool_cm.__enter__()
            # qkv3: [r, (i, ch)] with ch = [q(0:64) | qa(64:128) | v(128:192)]
            # k_dm: [r, (d, j)]  d-major so the Gram moving operand is contiguous
            qkv3 = qkv_pool.tile([P, N * 192], BF16, name="qkv3")
            qkv_r = qkv3.rearrange("p (i c) -> p i c", c=192)
            k_dm = qkv_pool.tile([P, D * N], BF16, name="k_dm")
            k_dm_r = k_dm.rearrange("p (d j) -> p d j", j=N)

            # ---------------- Phase 1: projections ----------------
            IB = 16  # i-block streamed per DMA
            for _p1 in range(phase_reps[0]):
                with (
                    tc.tile_pool(name="xt", bufs=2) as xt_pool,
                    tc.tile_pool(name="ps1", bufs=2, space="PSUM") as ps1_pool,
                ):
                    for ib in range(N // IB):
                        xts = []
                        for kt in range(KT):
                            t = xt_pool.tile(
                                [P, IB * P], BF16, name=f"xt{kt}", tag=f"xt{kt}"
                            )
                            nc.sync.dma_start(
                                t,
                                xT_ap[
                                    kt * P : (kt + 1) * P, ib * IB : (ib + 1) * IB, :
                                ].rearrange("e i r -> e (i r)"),
                            )
                            xts.append(t)
                        for ii in range(0, IB, 8):
                            ps = ps1_pool.tile([P, 2048], FP32, name="ps1", tag="ps1")
                            ps_r = ps.rearrange("p (i c) -> p i c", c=256)
                            for oct_ in range(8):
                                i_loc = ii + oct_
                                for kt in range(KT):
                                    nc.tensor.matmul(
                                        ps[:, oct_ * 256 : (oct_ + 1) * 256],
                                        lhsT=xts[kt][:, i_loc * P : (i_loc + 1) * P],
                                        rhs=w4_sb[:, kt],
                                        start=(kt == 0),
                                        stop=(kt == KT - 1),
                                    )
                            i_glob = ib * IB + ii
                            nc.vector.tensor_copy(
                                out=qkv_r[:, i_glob : i_glob + 8, :],
                                in_=ps_r[:, :, 0:192],
                            )
                            nc.vector.tensor_copy(
                                out=k_dm_r[:, :, i_glob : i_glob + 8].rearrange(
                                    "p d i -> p i d"
                                ),
                                in_=ps_r[:, :, 192:256],
                            )

            # ---------------- Phase 2: Gram contractions + v transpose ----------------
            # v_T[jc]: [j (128), (d, r)] bf16 for the context matmul
            v_T = [
                big_pool.tile([P, R * D], BF16, name=f"v_T{jc}") for jc in range(2)
            ]
            attn_sb = [
                sm_pool.tile([P, N], FP32, name=f"attn_sb{ic}", tag=f"attn_sb{ic}")
                for ic in range(2)
            ]
            a_sb = [
                sm_pool.tile([P, N], FP32, name=f"a_sb{ic}", tag=f"a_sb{ic}")
                for ic in range(2)
            ]

            for _p2 in range(phase_reps[1]):
                with (
                    tc.tile_pool(name="psacc", bufs=1, space="PSUM") as psacc_pool,
                    tc.tile_pool(name="pst", bufs=3, space="PSUM") as pst_pool,
                ):
                    a_ps = [
                        psacc_pool.tile([P, N], FP32, name=f"a_ps{ic}", tag=f"a_ps{ic}")
                        for ic in range(2)
                    ]
                    attn_ps = [
                        psacc_pool.tile([P, N], FP32, name=f"at_ps{ic}", tag=f"at_ps{ic}")
                        for ic in range(2)
                    ]
                    # a-channel first so the AllReduce can start early
                    for d in range(D):
                        kr = k_dm_r[:, d]
                        for ic in range(2):
                            nc.tensor.matmul(
                                a_ps[ic],
                                lhsT=qkv_r[:, ic * P : (ic + 1) * P, 64 + d],
                                rhs=kr,
                                start=(d == 0),
                                stop=(d == D - 1),
                            )
                    for ic in range(2):
                        nc.vector.tensor_copy(out=a_sb[ic], in_=a_ps[ic])
                        nc.sync.dma_start(ar_in[ic * P : (ic + 1) * P, :], a_sb[ic])
                    ar_out = dram_pool.tile(
                        [N, N], FP32, name="ar_out", addr_space="Shared",
                        tag=f"ar_out{_p2}",
                    )
                    nc.gpsimd.collective_compute(
                        "AllReduce",
                        mybir.AluOpType.add,
                        replica_groups=rg,
                        ins=[ar_in.opt()],
                        outs=[ar_out.opt()],
                    )

                    # attn channel
                    for d in range(D):
                        kr = k_dm_r[:, d]
                        for ic in range(2):
                            nc.tensor.matmul(
                                attn_ps[ic],
                                lhsT=qkv_r[:, ic * P : (ic + 1) * P, d],
                                rhs=kr,
                                start=(d == 0),
                                stop=(d == D - 1),
                            )
                    for ic in range(2):
                        nc.vector.tensor_copy(out=attn_sb[ic], in_=attn_ps[ic])

                    # v transposes: qkv [r, (j, 128+d)] -> v_T[jc] [j, (d, r)]
                    for jc in range(2):
                        vtr = v_T[jc].rearrange("p (d r) -> p d r", r=R)
                        for d in range(D):
                            tp = pst_pool.tile([P, P], BF16, name="tp", tag="tp")
                            nc.tensor.transpose(
                                tp, qkv_r[:, jc * P : (jc + 1) * P, 128 + d], ident
                            )
                            nc.vector.tensor_copy(out=vtr[:, d], in_=tp)
            qkv_pool_cm.__exit__(None, None, None)

            # ---------------- Phase 3: network branch + softmax ----------------
            probs_bf = [
                big_pool.tile([P, N], BF16, name=f"probs_bf{ic}") for ic in range(2)
            ]
            for _p3 in range(phase_reps[2]):
                with tc.tile_pool(name="ph3", bufs=1) as ph3:
                    a_full = [
                        ph3.tile([P, N], FP32, name=f"a_full{ic}", tag=f"a_full{ic}")
                        for ic in range(2)
                    ]
                    for ic in range(2):
                        nc.sync.dma_start(a_full[ic], ar_out[ic * P : (ic + 1) * P, :])
                    for ic in range(2):
                        net_sb = ph3.tile([P, N * M], FP32, name="net_sb", tag="net_sb")
                        nc.sync.dma_start(
                            net_sb,
                            net.ap()[ic * P : (ic + 1) * P].rearrange("i j m -> i (j m)"),
                        )
                        net_r = net_sb.rearrange("p (j m) -> p j m", m=M)
                        na = ph3.tile([P, N * M], FP32, name="na", tag="na")
                        na_r = na.rearrange("p (j m) -> p j m", m=M)
                        a_b = a_full[ic][:, :, None].to_broadcast([P, N, M])
                        nc.vector.tensor_tensor(na_r, net_r, a_b, mybir.AluOpType.mult)
                        # softmax over m (4 channels); |na| << 1 so no max-shift needed
                        ne = ph3.tile([P, N * M], FP32, name="ne", tag="ne")
                        nc.scalar.activation(
                            ne, na, mybir.ActivationFunctionType.Exp
                        )
                        ne_r = ne.rearrange("p (j m) -> p j m", m=M)
                        s4 = ph3.tile([P, N], FP32, name="s4", tag="s4")
                        nc.vector.reduce_sum(s4, ne_r, axis=mybir.AxisListType.X)
                        rinv = ph3.tile([P, N], FP32, name="rinv", tag="rinv")
                        nc.vector.reciprocal(rinv, s4)
                        # net_bias = (sum_m network * exp) / sum_m exp
                        tw = ph3.tile([P, N * M], FP32, name="tw", tag="tw")
                        nc.vector.tensor_tensor(tw, ne, net_sb, mybir.AluOpType.mult)
                        tw_r = tw.rearrange("p (j m) -> p j m", m=M)
                        nb = ph3.tile([P, N], FP32, name="nb", tag="nb")
                        nc.vector.reduce_sum(nb, tw_r, axis=mybir.AxisListType.X)
                        nc.vector.tensor_tensor(nb, nb, rinv, mybir.AluOpType.mult)
                        # attn_final = attn + l * net_bias + negeye
                        lnb = ph3.tile([P, N], FP32, name="lnb", tag="lnb")
                        nc.vector.tensor_scalar_mul(lnb, nb, lrep_sb[:, 0:1])
                        negeye_sb = ph3.tile([P, N], FP32, name="negeye_sb", tag="ney")
                        nc.sync.dma_start(negeye_sb, negeye.ap()[ic])
                        af = ph3.tile([P, N], FP32, name="af", tag="af")
                        nc.vector.tensor_tensor(af, attn_sb[ic], lnb, mybir.AluOpType.add)
                        nc.vector.tensor_tensor(af, af, negeye_sb, mybir.AluOpType.add)
                        # row softmax over j
                        negmx = ph3.tile([P, 1], FP32, name="negmx", tag="negmx")
                        nc.vector.tensor_reduce(
                            negmx, af, axis=mybir.AxisListType.X,
                            op=mybir.AluOpType.max, negate=True,
                        )
                        pex = ph3.tile([P, N], FP32, name="pex", tag="pex")
                        rowsum = ph3.tile([P, 1], FP32, name="rowsum", tag="rowsum")
                        nc.scalar.activation(
                            pex, af, mybir.ActivationFunctionType.Exp,
                            bias=negmx[:, 0:1], accum_out=rowsum[:, 0:1],
                        )
                        rinv2 = ph3.tile([P, 1], FP32, name="rinv2", tag="rinv2")
                        nc.vector.reciprocal(rinv2, rowsum)
                        probs_f = ph3.tile([P, N], FP32, name="probs_f", tag="probs_f")
                        nc.vector.tensor_scalar_mul(probs_f, pex, rinv2[:, 0:1])
                        nc.sync.dma_start(
                            probs_out.ap()[ic * P : (ic + 1) * P, :], probs_f
                        )
                        nc.vector.tensor_copy(out=probs_bf[ic], in_=probs_f)

            # ---------------- Phase 4: probs transpose + context ----------------
            for _p4 in range(phase_reps[3]):
                with (
                    tc.tile_pool(name="ph4", bufs=2) as ph4,
                    tc.tile_pool(name="pst2", bufs=2, space="PSUM") as pst2_pool,
                    tc.tile_pool(name="psc", bufs=6, space="PSUM") as psc_pool,
                ):
                    probsT = [
                        ph4.tile([P, N], BF16, name=f"probsT{jc}", tag=f"probsT{jc}",
                                 bufs=1)
                        for jc in range(2)
                    ]
                    for jc in range(2):
                        for ic in range(2):
                            tp2 = pst2_pool.tile([P, P], BF16, name="tp2", tag="tp2")
                            nc.tensor.transpose(
                                tp2, probs_bf[ic][:, jc * P : (jc + 1) * P], ident
                            )
                            nc.vector.tensor_copy(
                                out=probsT[jc][:, ic * P : (ic + 1) * P], in_=tp2
                            )
                    for d in range(D):
                        cps = psc_pool.tile([P, N], FP32, name="cps", tag="cps")
                        for jc in range(2):
                            nc.tensor.matmul(
                                cps,
                                lhsT=v_T[jc][:, d * P : (d + 1) * P],
                                rhs=probsT[jc],
                                start=(jc == 0),
                                stop=(jc == 1),
                            )
                        cs = ph4.tile([P, N], BF16, name="cs", tag="cs", bufs=4)
                        nc.vector.tensor_copy(out=cs, in_=cps)
                        nc.sync.dma_start(a2a_in[:, d, :], cs)

            # ---------------- Phase 5: AllToAll ----------------
            for _p5 in range(phase_reps[4]):
                a2a_out = dram_pool.tile(
                    [NCORES, RS, D, N], BF16, name="a2a_out", tag=f"a2a_out{_p5}"
                )
                nc.gpsimd.collective_compute(
                    "AllToAll",
                    mybir.AluOpType.bypass,
                    replica_groups=rg,
                    ins=[a2a_in.opt()],
                    outs=[a2a_out.opt()],
                )

            # ---------------- Phase 6: output projection ----------------
            for _p6 in range(phase_reps[5]):
                with (
                    tc.tile_pool(name="ph6", bufs=1) as ph6,
                    tc.tile_pool(name="ps6", bufs=4, space="PSUM") as ps6_pool,
                    tc.tile_pool(name="ph6o", bufs=4) as ph6o,
                ):
                    # gathered context, e-major: partition = (h-pair, d)
                    ctx_sb = []
                    for kt in range(KT):
                        t = ph6.tile([P, RS, N], BF16, name=f"ctx_sb{kt}")
                        for hh in range(2):
                            for rh in range(4):
                                nc.sync.dma_start(
                                    t[hh * D : (hh + 1) * D, rh * 4 : (rh + 1) * 4],
                                    a2a_out[2 * kt + hh][
                                        rh * 4 : (rh + 1) * 4
                                    ].rearrange("r d i -> d r i"),
                                )
                        ctx_sb.append(t)
                    for r_loc in range(RS):
                        for ih in range(2):
                            ps6 = ps6_pool.tile([P, E], FP32, name="ps6", tag="ps6")
                            for kt in range(KT):
                                nc.tensor.matmul(
                                    ps6,
                                    lhsT=ctx_sb[kt][:, r_loc, ih * P : (ih + 1) * P],
                                    rhs=wo_sb[:, kt],
                                    start=(kt == 0),
                                    stop=(kt == KT - 1),
                                )
                            osb = ph6o.tile([P, E], FP32, name="osb", tag="osb")
                            nc.vector.tensor_copy(out=osb, in_=ps6)
                            row0 = r_loc * N + ih * P
                            nc.sync.dma_start(
                                out_slice.ap()[row0 : row0 + P, :], osb
                            )

    nc.compile()
    return nc


_CACHE = {}


def _get_program():
    if "nc" not in _CACHE:
        _CACHE["nc"] = build_program()
    return _CACHE["nc"]


def _make_in_maps(x, network, Wq, bq, Wk, bk, Wv, bv, Wo, bo, Wq1, bq1, Wk1, bk1, l):
    x = np.asarray(x, np.float32)
    network = np.asarray(network, np.float32)
    for b_, nm in ((bq, "bq"), (bk, "bk"), (bv, "bv"), (bo, "bo"),
                   (bq1, "bq1"), (bk1, "bk1")):
        assert np.allclose(np.asarray(b_), 0.0), f"nonzero bias {nm} unsupported"
    wc = np.asarray(Wq1, np.float64) @ np.asarray(bk1, np.float64)
    assert np.allclose(wc, 0.0)

    # wa folds the whole q1_proj/net-k MLP into one weighted d-contraction.
    wa = (np.asarray(Wq1, np.float64) @ np.asarray(Wk1, np.float64)[0]) / H  # [D]

    xTn = np.ascontiguousarray(
        x[:, :, 0, :].transpose(2, 1, 0)
    ).astype(BF16_NP)  # [E, N, R]

    negeye = np.zeros((2, P, N), np.float32)
    for ic in range(2):
        for p in range(P):
            negeye[ic, p, ic * P + p] = NEG

    netn = np.ascontiguousarray(network[0], np.float32)  # [N, N, M]
    Wo_n = np.ascontiguousarray(np.asarray(Wo, np.float32)).astype(BF16_NP)

    in_maps = []
    for h in range(NCORES):
        sl = slice(h * D, (h + 1) * D)
        # column order must match the kernel: [q | qa | v | k]
        w4v = np.empty((E, 256), np.float64)
        w4v[:, 0:64] = np.asarray(Wq, np.float64)[:, sl] * SCALING
        w4v[:, 64:128] = w4v[:, 0:64] * wa[None, :]
        w4v[:, 128:192] = np.asarray(Wv, np.float64)[:, sl]
        w4v[:, 192:256] = np.asarray(Wk, np.float64)[:, sl]
        in_maps.append(
            {
                "xT": xTn,
                "w4": w4v.astype(BF16_NP),
                "wo_t": Wo_n,
                "lrep": np.full((P, 1), np.float32(np.asarray(l)[h, 0, 0, 0]),
                                np.float32),
                "negeye": negeye,
                "net": netn,
            }
        )
    return in_maps


def _assemble(results, l):
    out = np.empty((R, N, B, E), np.float32)
    probs = np.empty((H, B, N, N), np.float32)
    for h in range(NCORES):
        res = results[h]
        out[h * RS : (h + 1) * RS, :, 0, :] = res["out_slice"].reshape(RS, N, E)
        probs[h, 0] = res["probs_out"]
    return out, probs, np.asarray(l, np.float32)


def kernel(**inputs):
    nc = _get_program()
    in_maps = _make_in_maps(**inputs)
    r = bass_utils.run_bass_kernel_spmd(nc, in_maps, core_ids=list(range(NCORES)))
    return _assemble(r.results, inputs["l"])


# revision 23
# speedup vs baseline: 14.0001x; 1.0284x over previous
# Trainium2 Bass kernel for nn_Attention_10342281248904 (sparse_attention).
#
# Sharding: tensor-parallel over heads H=8, one head per NeuronCore.
# Each core: q/k/v projections for its head, the hnijd Gram contraction,
# softmax + network-bias branch, context matmul. The mean-over-heads in
# q1_proj is algebraically collapsed to a single weighted d-contraction
# ("a" channel) and realized with one AllReduce; the output projection is
# done after an AllToAll that gives each core a 16-row slice of the full
# 512-channel context (row-sliced data parallel out_proj, no final
# all-reduce needed).
import math
import os
import numpy as np
import ml_dtypes

import concourse.bass as bass
import concourse.mybir as mybir
import concourse.tile as tile
from concourse import bacc
from concourse import bass_utils
from concourse.masks import make_identity

# Problem constants (hardcoded per task contract)
R, N, B, E, H, M = 128, 256, 1, 512, 8, 4
D = E // H          # 64 head dim
NCORES = 8
RS = R // NCORES    # 16 rows of R per core in the output slice
P = 128
KT = E // P         # 4 contraction tiles for E
NEG = -1.0e9
SCALING = (D ** -0.5) / math.sqrt(R)

FP32 = mybir.dt.float32
BF16 = mybir.dt.bfloat16

BF16_NP = ml_dtypes.bfloat16


def build_program(body_reps=1, phase_reps=(1, 1, 1, 1, 1, 1)):
    """Build the SPMD Bass program (same NEFF on all 8 cores; per-core
    behavior differs only through per-core input tensors).

    body_reps repeats the whole body; phase_reps[i] repeats phase i+1
    (every phase is idempotent) — both only used for slope timing."""
    nc = bacc.Bacc(
        "TRN2",
        target_bir_lowering=False,
        debug=False,
        num_devices=NCORES,
    )

    # ---- I/O ----
    xT = nc.dram_tensor("xT", [E, N, R], BF16, kind="ExternalInput")
    w4 = nc.dram_tensor("w4", [E, 256], BF16, kind="ExternalInput")
    wo_t = nc.dram_tensor("wo_t", [E, E], BF16, kind="ExternalInput")
    lrep = nc.dram_tensor("lrep", [P, 1], FP32, kind="ExternalInput")
    negeye = nc.dram_tensor("negeye", [2, P, N], FP32, kind="ExternalInput")
    net = nc.dram_tensor("net", [N, N, M], FP32, kind="ExternalInput")

    probs_out = nc.dram_tensor("probs_out", [N, N], FP32, kind="ExternalOutput")
    out_slice = nc.dram_tensor("out_slice", [RS * N, E], FP32, kind="ExternalOutput")

    xT_ap = xT.ap()
    rg = [list(range(NCORES))]

    with tile.TileContext(nc) as tc:
      for _rep in range(body_reps):
        with (
            tc.tile_pool(name="const", bufs=1) as const_pool,
            tc.tile_pool(name="big", bufs=1) as big_pool,
            tc.tile_pool(name="dram", bufs=1, space="DRAM") as dram_pool,
            tc.tile_pool(name="sm", bufs=2) as sm_pool,
        ):
            # Persistent SBUF tensors
            w4_sb = const_pool.tile([P, KT, 256], BF16, name="w4_sb")
            nc.sync.dma_start(w4_sb, w4.ap().rearrange("(kt p) c -> p kt c", p=P))
            ident = const_pool.tile([P, P], BF16, name="ident")
            make_identity(nc, ident)
            lrep_sb = const_pool.tile([P, 1], FP32, name="lrep_sb")
            nc.sync.dma_start(lrep_sb, lrep.ap())
            wo_sb = const_pool.tile([P, KT, E], BF16, name="wo_sb")
            nc.sync.dma_start(
                wo_sb, wo_t.ap().rearrange("(kt p) c -> p kt c", p=P)
            )

            # DRAM bounce buffers for collectives
            ar_in = dram_pool.tile([N, N], FP32, name="ar_in")
            a2a_in = dram_pool.tile([R, D, N], BF16, name="a2a_in")

            # ---------------- Phases 1+2 share the big qkv tensors ----------------
            qkv_pool_cm = tc.tile_pool(name="qkvp", bufs=1)
            qkv_pool = qkv_pool_cm.__enter__()
            # qkv3: [r, (i, ch)] with ch = [q(0:64) | qa(64:128) | v(128:192)]
            # k_dm: [r, (d, j)]  d-major so the Gram moving operand is contiguous
            qkv3 = qkv_pool.tile([P, N * 192], BF16, name="qkv3")
            qkv_r = qkv3.rearrange("p (i c) -> p i c", c=192)
            k_dm = qkv_pool.tile([P, D * N], BF16, name="k_dm")
            k_dm_r = k_dm.rearrange("p (d j) -> p d j", j=N)

            # ---------------- Phase 1: projections ----------------
            IB = 16  # i-block streamed per DMA
            for _p1 in range(phase_reps[0]):
                with (
                    tc.tile_pool(name="xt", bufs=2) as xt_pool,
                    tc.tile_pool(name="ps1", bufs=2, space="PSUM") as ps1_pool,
                ):
                    dma_engines = [nc.sync, nc.gpsimd, nc.scalar, nc.sync]
                    for ib in range(N // IB):
                        xts = []
                        for kt in range(KT):
                            t = xt_pool.tile(
                                [P, IB * P], BF16, name=f"xt{kt}", tag=f"xt{kt}"
                            )
                            dma_engines[kt].dma_start(
                                t,
                                xT_ap[
                                    kt * P : (kt + 1) * P, ib * IB : (ib + 1) * IB, :
                                ].rearrange("e i r -> e (i r)"),
                            )
                            xts.append(t)
                        for ii in range(0, IB, 8):
                            ps = ps1_pool.tile([P, 2048], FP32, name="ps1", tag="ps1")
                            ps_r = ps.rearrange("p (i c) -> p i c", c=256)
                            for oct_ in range(8):
                                i_loc = ii + oct_
                                for kt in range(KT):
                                    nc.tensor.matmul(
                                        ps[:, oct_ * 256 : (oct_ + 1) * 256],
                                        lhsT=xts[kt][:, i_loc * P : (i_loc + 1) * P],
                                        rhs=w4_sb[:, kt],
                                        start=(kt == 0),
                                        stop=(kt == KT - 1),
                                    )
                            i_glob = ib * IB + ii
                            nc.vector.tensor_copy(
                                out=qkv_r[:, i_glob : i_glob + 8, :],
                                in_=ps_r[:, :, 0:192],
                            )
                            nc.vector.tensor_copy(
                                out=k_dm_r[:, :, i_glob : i_glob + 8].rearrange(
                                    "p d i -> p i d"
                                ),
                                in_=ps_r[:, :, 192:256],
                            )

            # ---------------- Phase 2: Gram contractions + v transpose ----------------
            # v_T[jc]: [j (128), (d, r)] bf16 for the context matmul
            v_T = [
                big_pool.tile([P, R * D], BF16, name=f"v_T{jc}") for jc in range(2)
            ]
            attn_sb = [
                sm_pool.tile([P, N], FP32, name=f"attn_sb{ic}", tag=f"attn_sb{ic}")
                for ic in range(2)
            ]
            a_sb = [
                sm_pool.tile([P, N], FP32, name=f"a_sb{ic}", tag=f"a_sb{ic}")
                for ic in range(2)
            ]

            for _p2 in range(phase_reps[1]):
                with (
                    tc.tile_pool(name="psacc", bufs=1, space="PSUM") as psacc_pool,
                    tc.tile_pool(name="pst", bufs=3, space="PSUM") as pst_pool,
                ):
                    a_ps = [
                        psacc_pool.tile([P, N], FP32, name=f"a_ps{ic}", tag=f"a_ps{ic}")
                        for ic in range(2)
                    ]
                    attn_ps = [
                        psacc_pool.tile([P, N], FP32, name=f"at_ps{ic}", tag=f"at_ps{ic}")
                        for ic in range(2)
                    ]
                    # a-channel first so the AllReduce can start early
                    for d in range(D):
                        kr = k_dm_r[:, d]
                        for ic in range(2):
                            nc.tensor.matmul(
                                a_ps[ic],
                                lhsT=qkv_r[:, ic * P : (ic + 1) * P, 64 + d],
                                rhs=kr,
                                start=(d == 0),
                                stop=(d == D - 1),
                            )
                    for ic in range(2):
                        nc.vector.tensor_copy(out=a_sb[ic], in_=a_ps[ic])
                        nc.sync.dma_start(ar_in[ic * P : (ic + 1) * P, :], a_sb[ic])
                    ar_out = dram_pool.tile(
                        [N, N], FP32, name="ar_out", addr_space="Shared",
                        tag=f"ar_out{_p2}",
                    )
                    nc.gpsimd.collective_compute(
                        "AllReduce",
                        mybir.AluOpType.add,
                        replica_groups=rg,
                        ins=[ar_in.opt()],
                        outs=[ar_out.opt()],
                    )

                    # attn channel
                    for d in range(D):
                        kr = k_dm_r[:, d]
                        for ic in range(2):
                            nc.tensor.matmul(
                                attn_ps[ic],
                                lhsT=qkv_r[:, ic * P : (ic + 1) * P, d],
                                rhs=kr,
                                start=(d == 0),
                                stop=(d == D - 1),
                            )
                    for ic in range(2):
                        nc.vector.tensor_copy(out=attn_sb[ic], in_=attn_ps[ic])

                    # v transposes: qkv [r, (j, 128+d)] -> v_T[jc] [j, (d, r)]
                    for jc in range(2):
                        vtr = v_T[jc].rearrange("p (d r) -> p d r", r=R)
                        for d in range(D):
                            tp = pst_pool.tile([P, P], BF16, name="tp", tag="tp")
                            nc.tensor.transpose(
                                tp, qkv_r[:, jc * P : (jc + 1) * P, 128 + d], ident
                            )
                            nc.vector.tensor_copy(out=vtr[:, d], in_=tp)
            qkv_pool_cm.__exit__(None, None, None)

            # ---------------- Phase 3: network branch + softmax ----------------
            probs_bf = [
                big_pool.tile([P, N], BF16, name=f"probs_bf{ic}") for ic in range(2)
            ]
            for _p3 in range(phase_reps[2]):
                with tc.tile_pool(name="ph3", bufs=1) as ph3:
                    a_full = [
                        ph3.tile([P, N], FP32, name=f"a_full{ic}", tag=f"a_full{ic}")
                        for ic in range(2)
                    ]
                    for ic in range(2):
                        nc.sync.dma_start(a_full[ic], ar_out[ic * P : (ic + 1) * P, :])
                    for ic in range(2):
                        net_sb = ph3.tile([P, N * M], FP32, name="net_sb", tag="net_sb")
                        nc.sync.dma_start(
                            net_sb,
                            net.ap()[ic * P : (ic + 1) * P].rearrange("i j m -> i (j m)"),
                        )
                        net_r = net_sb.rearrange("p (j m) -> p j m", m=M)
                        na = ph3.tile([P, N * M], FP32, name="na", tag="na")
                        na_r = na.rearrange("p (j m) -> p j m", m=M)
                        a_b = a_full[ic][:, :, None].to_broadcast([P, N, M])
                        nc.vector.tensor_tensor(na_r, net_r, a_b, mybir.AluOpType.mult)
                        # softmax over m (4 channels); |na| << 1 so no max-shift needed
                        ne = ph3.tile([P, N * M], FP32, name="ne", tag="ne")
                        nc.scalar.activation(
                            ne, na, mybir.ActivationFunctionType.Exp
                        )
                        ne_r = ne.rearrange("p (j m) -> p j m", m=M)
                        s4 = ph3.tile([P, N], FP32, name="s4", tag="s4")
                        nc.vector.reduce_sum(s4, ne_r, axis=mybir.AxisListType.X)
                        rinv = ph3.tile([P, N], FP32, name="rinv", tag="rinv")
                        nc.vector.reciprocal(rinv, s4)
                        # net_bias = (sum_m network * exp) / sum_m exp
                        tw = ph3.tile([P, N * M], FP32, name="tw", tag="tw")
                        nc.vector.tensor_tensor(tw, ne, net_sb, mybir.AluOpType.mult)
                        tw_r = tw.rearrange("p (j m) -> p j m", m=M)
                        nb = ph3.tile([P, N], FP32, name="nb", tag="nb")
                        nc.vector.reduce_sum(nb, tw_r, axis=mybir.AxisListType.X)
                        nc.vector.tensor_tensor(nb, nb, rinv, mybir.AluOpType.mult)
                        # attn_final = attn + l * net_bias + negeye
                        lnb = ph3.tile([P, N], FP32, name="lnb", tag="lnb")
                        nc.vector.tensor_scalar_mul(lnb, nb, lrep_sb[:, 0:1])
                        negeye_sb = ph3.tile([P, N], FP32, name="negeye_sb", tag="ney")
                        nc.sync.dma_start(negeye_sb, negeye.ap()[ic])
                        af = ph3.tile([P, N], FP32, name="af", tag="af")
                        nc.vector.tensor_tensor(af, attn_sb[ic], lnb, mybir.AluOpType.add)
                        nc.vector.tensor_tensor(af, af, negeye_sb, mybir.AluOpType.add)
                        # row softmax over j
                        negmx = ph3.tile([P, 1], FP32, name="negmx", tag="negmx")
                        nc.vector.tensor_reduce(
                            negmx, af, axis=mybir.AxisListType.X,
                            op=mybir.AluOpType.max, negate=True,
                        )
                        pex = ph3.tile([P, N], FP32, name="pex", tag="pex")
                        rowsum = ph3.tile([P, 1], FP32, name="rowsum", tag="rowsum")
                        nc.scalar.activation(
                            pex, af, mybir.ActivationFunctionType.Exp,
                            bias=negmx[:, 0:1], accum_out=rowsum[:, 0:1],
                        )
                        rinv2 = ph3.tile([P, 1], FP32, name="rinv2", tag="rinv2")
                        nc.vector.reciprocal(rinv2, rowsum)
                        probs_f = ph3.tile([P, N], FP32, name="probs_f", tag="probs_f")
                        nc.vector.tensor_scalar_mul(probs_f, pex, rinv2[:, 0:1])
                        nc.sync.dma_start(
                            probs_out.ap()[ic * P : (ic + 1) * P, :], probs_f
                        )
                        nc.vector.tensor_copy(out=probs_bf[ic], in_=probs_f)

            # ---------------- Phase 4: probs transpose + context ----------------
            for _p4 in range(phase_reps[3]):
                with (
                    tc.tile_pool(name="ph4", bufs=2) as ph4,
                    tc.tile_pool(name="pst2", bufs=2, space="PSUM") as pst2_pool,
                    tc.tile_pool(name="psc", bufs=6, space="PSUM") as psc_pool,
                ):
                    probsT = [
                        ph4.tile([P, N], BF16, name=f"probsT{jc}", tag=f"probsT{jc}",
                                 bufs=1)
                        for jc in range(2)
                    ]
                    for jc in range(2):
                        for ic in range(2):
                            tp2 = pst2_pool.tile([P, P], BF16, name="tp2", tag="tp2")
                            nc.tensor.transpose(
                                tp2, probs_bf[ic][:, jc * P : (jc + 1) * P], ident
                            )
                            nc.vector.tensor_copy(
                                out=probsT[jc][:, ic * P : (ic + 1) * P], in_=tp2
                            )
                    for d in range(D):
                        cps = psc_pool.tile([P, N], FP32, name="cps", tag="cps")
                        for jc in range(2):
                            nc.tensor.matmul(
                                cps,
                                lhsT=v_T[jc][:, d * P : (d + 1) * P],
                                rhs=probsT[jc],
                                start=(jc == 0),
                                stop=(jc == 1),
                            )
                        cs = ph4.tile([P, N], BF16, name="cs", tag="cs", bufs=4)
                        nc.vector.tensor_copy(out=cs, in_=cps)
                        [nc.sync, nc.gpsimd, nc.scalar][d % 3].dma_start(
                            a2a_in[:, d, :], cs
                        )

            # ---------------- Phase 5: AllToAll ----------------
            for _p5 in range(phase_reps[4]):
                a2a_out = dram_pool.tile(
                    [NCORES, RS, D, N], BF16, name="a2a_out", tag=f"a2a_out{_p5}"
                )
                nc.gpsimd.collective_compute(
                    "AllToAll",
                    mybir.AluOpType.bypass,
                    replica_groups=rg,
                    ins=[a2a_in.opt()],
                    outs=[a2a_out.opt()],
                )

            # ---------------- Phase 6: output projection ----------------
            for _p6 in range(phase_reps[5]):
                with (
                    tc.tile_pool(name="ph6", bufs=1) as ph6,
                    tc.tile_pool(name="ps6", bufs=4, space="PSUM") as ps6_pool,
                    tc.tile_pool(name="ph6o", bufs=4) as ph6o,
                ):
                    # gathered context, e-major: partition = (h-pair, d)
                    ctx_sb = []
                    eng6 = [nc.sync, nc.gpsimd, nc.scalar, nc.sync]
                    for kt in range(KT):
                        t = ph6.tile([P, RS, N], BF16, name=f"ctx_sb{kt}")
                        for hh in range(2):
                            for rh in range(4):
                                eng6[(kt * 8 + hh * 4 + rh) % 4].dma_start(
                                    t[hh * D : (hh + 1) * D, rh * 4 : (rh + 1) * 4],
                                    a2a_out[2 * kt + hh][
                                        rh * 4 : (rh + 1) * 4
                                    ].rearrange("r d i -> d r i"),
                                )
                        ctx_sb.append(t)
                    for r_loc in range(RS):
                        for ih in range(2):
                            ps6 = ps6_pool.tile([P, E], FP32, name="ps6", tag="ps6")
                            for kt in range(KT):
                                nc.tensor.matmul(
                                    ps6,
                                    lhsT=ctx_sb[kt][:, r_loc, ih * P : (ih + 1) * P],
                                    rhs=wo_sb[:, kt],
                                    start=(kt == 0),
                                    stop=(kt == KT - 1),
                                )
                            osb = ph6o.tile([P, E], FP32, name="osb", tag="osb")
                            nc.vector.tensor_copy(out=osb, in_=ps6)
                            row0 = r_loc * N + ih * P
                            nc.sync.dma_start(
                                out_slice.ap()[row0 : row0 + P, :], osb
                            )

    nc.compile()
    return nc


_CACHE = {}


def _get_program():
    if "nc" not in _CACHE:
        _CACHE["nc"] = build_program()
    return _CACHE["nc"]


def _make_in_maps(x, network, Wq, bq, Wk, bk, Wv, bv, Wo, bo, Wq1, bq1, Wk1, bk1, l):
    x = np.asarray(x, np.float32)
    network = np.asarray(network, np.float32)
    for b_, nm in ((bq, "bq"), (bk, "bk"), (bv, "bv"), (bo, "bo"),
                   (bq1, "bq1"), (bk1, "bk1")):
        assert np.allclose(np.asarray(b_), 0.0), f"nonzero bias {nm} unsupported"
    wc = np.asarray(Wq1, np.float64) @ np.asarray(bk1, np.float64)
    assert np.allclose(wc, 0.0)

    # wa folds the whole q1_proj/net-k MLP into one weighted d-contraction.
    wa = (np.asarray(Wq1, np.float64) @ np.asarray(Wk1, np.float64)[0]) / H  # [D]

    xTn = np.ascontiguousarray(
        x[:, :, 0, :].transpose(2, 1, 0)
    ).astype(BF16_NP)  # [E, N, R]

    negeye = np.zeros((2, P, N), np.float32)
    for ic in range(2):
        for p in range(P):
            negeye[ic, p, ic * P + p] = NEG

    netn = np.ascontiguousarray(network[0], np.float32)  # [N, N, M]
    Wo_n = np.ascontiguousarray(np.asarray(Wo, np.float32)).astype(BF16_NP)

    in_maps = []
    for h in range(NCORES):
        sl = slice(h * D, (h + 1) * D)
        # column order must match the kernel: [q | qa | v | k]
        w4v = np.empty((E, 256), np.float64)
        w4v[:, 0:64] = np.asarray(Wq, np.float64)[:, sl] * SCALING
        w4v[:, 64:128] = w4v[:, 0:64] * wa[None, :]
        w4v[:, 128:192] = np.asarray(Wv, np.float64)[:, sl]
        w4v[:, 192:256] = np.asarray(Wk, np.float64)[:, sl]
        in_maps.append(
            {
                "xT": xTn,
                "w4": w4v.astype(BF16_NP),
                "wo_t": Wo_n,
                "lrep": np.full((P, 1), np.float32(np.asarray(l)[h, 0, 0, 0]),
                                np.float32),
                "negeye": negeye,
                "net": netn,
            }
        )
    return in_maps


def _assemble(results, l):
    out = np.empty((R, N, B, E), np.float32)
    probs = np.empty((H, B, N, N), np.float32)
    for h in range(NCORES):
        res = results[h]
        out[h * RS : (h + 1) * RS, :, 0, :] = res["out_slice"].reshape(RS, N, E)
        probs[h, 0] = res["probs_out"]
    return out, probs, np.asarray(l, np.float32)


def kernel(**inputs):
    nc = _get_program()
    in_maps = _make_in_maps(**inputs)
    r = bass_utils.run_bass_kernel_spmd(nc, in_maps, core_ids=list(range(NCORES)))
    return _assemble(r.results, inputs["l"])
